# revision 1
# baseline (speedup 1.0000x reference)
"""Trainium2 Bass kernel for nn_MultiHeadAttention (RoPE MHA, B=2 S=2048 E=1024 H=16).

Sharding: tensor-parallel over heads — 2 heads per core on 8 cores. Each core
computes its heads' q/k/v projections, RoPE, attention, and the partial output
projection (its rows of Wo); the host sums the 8 partials and adds bo.

Per-core device program layouts:
  q/k as [d, token] (transposed) so attention scores come out as [ks, qs] and
  softmax's row-sum is obtained by appending a ones column to v (the Z row
  falls out of the same matmul that computes ctx). Normalization multiplies
  by 1/Z broadcast across partitions via a DRAM bounce. RoPE's rotate_half is
  a signed-permutation matmul on the tensor engine.
"""

import os
import sys
from contextlib import ExitStack

import numpy as np

for _p in ("/opt/trn_rl_repo", "/opt/pypackages"):
    if _p not in sys.path and os.path.isdir(_p):
        sys.path.append(_p)

import concourse.bass as bass
import concourse.mybir as mybir
import concourse.tile as tile
from concourse import bacc
from concourse import bass_utils

F32 = mybir.dt.float32
AF = mybir.ActivationFunctionType
OP = mybir.AluOpType

B = 2
S = 2048
E = 1024
H = 16
D = 64
N_CORES = 8
HPC = H // N_CORES  # heads per core = 2
HD = HPC * D  # 128

MM_MODE = os.environ.get("MHA_MM_MODE", "f32r")  # 'f32' | 'f32r' | 'bf16'

LAST_RESULTS = None  # BassKernelResults of the most recent run (for test harness)
_NC_CACHE = {}


def build_mha_nc(mm_mode):
    T = B * S
    TC = 512  # token chunk for projections
    NCH = T // TC
    QC = 512  # query chunk in attention
    NQC = S // QC
    NKT = S // 128  # key tiles per batch
    KE = E // 128  # contraction tiles for projections

    dt_in = mybir.dt.bfloat16 if mm_mode == "bf16" else F32

    def mmcast(ap):
        if mm_mode == "f32r":
            return ap.bitcast(mybir.dt.float32r)
        return ap

    nc = bacc.Bacc(None, target_bir_lowering=False, debug=False)

    xT = nc.dram_tensor("xT", [E, T], dt_in, kind="ExternalInput")
    wq = nc.dram_tensor("wq", [E, HD], dt_in, kind="ExternalInput")
    wk = nc.dram_tensor("wk", [E, HD], dt_in, kind="ExternalInput")
    wv = nc.dram_tensor("wv", [E, HD], dt_in, kind="ExternalInput")
    bq = nc.dram_tensor("bq", [HD, 1], F32, kind="ExternalInput")
    bk = nc.dram_tensor("bk", [HD, 1], F32, kind="ExternalInput")
    bv = nc.dram_tensor("bv", [1, HD], F32, kind="ExternalInput")
    wo0 = nc.dram_tensor("wo0", [D, E], dt_in, kind="ExternalInput")
    wo1 = nc.dram_tensor("wo1", [D, E], dt_in, kind="ExternalInput")
    cosT = nc.dram_tensor("cosT", [HD, T], F32, kind="ExternalInput")
    sinT = nc.dram_tensor("sinT", [HD, T], F32, kind="ExternalInput")
    rot = nc.dram_tensor("rot", [HD, HD], dt_in, kind="ExternalInput")
    yp = nc.dram_tensor("yp", [T, E], F32, kind="ExternalOutput")

    scale = 1.0 / np.sqrt(D)

    with tile.TileContext(nc) as tc, ExitStack() as ctx:
        const = ctx.enter_context(tc.tile_pool(name="const", bufs=1))
        xt_pool = ctx.enter_context(tc.tile_pool(name="xt", bufs=KE + 2))
        qkraw_pool = ctx.enter_context(tc.tile_pool(name="qkraw", bufs=3))
        rope_tmp = ctx.enter_context(tc.tile_pool(name="ropetmp", bufs=3))
        persist = ctx.enter_context(tc.tile_pool(name="persist", bufs=1))
        exps_pool = ctx.enter_context(tc.tile_pool(name="exps", bufs=4))
        zr_pool = ctx.enter_context(tc.tile_pool(name="zr", bufs=4))
        zb_pool = ctx.enter_context(tc.tile_pool(name="zb", bufs=4))
        osb_pool = ctx.enter_context(tc.tile_pool(name="osb", bufs=4))
        dram = ctx.enter_context(tc.tile_pool(name="dram", bufs=6, space="DRAM"))

        ps_acc = ctx.enter_context(tc.tile_pool(name="ps_acc", bufs=3, space="PSUM"))
        ps_mmt = ctx.enter_context(tc.tile_pool(name="ps_mmt", bufs=2, space="PSUM"))
        ps_ctx = ctx.enter_context(tc.tile_pool(name="ps_ctx", bufs=2, space="PSUM"))

        def load_const(name, dram_t, shape, dt):
            t = const.tile(shape, dt, name=name, tag=name)
            nc.sync.dma_start(t[:], dram_t.ap())
            return t

        wq_sb = [None] * KE
        wk_sb = [None] * KE
        wv_sb = [None] * KE
        for k in range(KE):
            for nm, dr, arr in (("wq", wq, wq_sb), ("wk", wk, wk_sb), ("wv", wv, wv_sb)):
                t = const.tile([128, HD], dt_in, name=f"{nm}_{k}", tag=f"{nm}_{k}")
                nc.sync.dma_start(t[:], dr.ap()[128 * k : 128 * (k + 1), :])
                arr[k] = t
        bq_sb = load_const("bq_sb", bq, [HD, 1], F32)
        bk_sb = load_const("bk_sb", bk, [HD, 1], F32)
        wo_sb = [
            load_const("wo0_sb", wo0, [D, E], dt_in),
            load_const("wo1_sb", wo1, [D, E], dt_in),
        ]
        rot_sb = load_const("rot_sb", rot, [HD, HD], dt_in)
        cos_sb = load_const("cos_sb", cosT, [HD, T], F32)
        sin_sb = load_const("sin_sb", sinT, [HD, T], F32)
        bvb_sb = const.tile([128, HD], F32, name="bvb_sb", tag="bvb_sb")
        nc.sync.dma_start(
            bvb_sb[:],
            bass.AP(tensor=bv.ap().tensor, offset=bv.ap().offset, ap=[[0, 128], [1, HD]]),
        )

        q_rope = persist.tile([HD, T], dt_in, name="q_rope", tag="q_rope")
        k_rope = persist.tile([HD, T], dt_in, name="k_rope", tag="k_rope")
        v_sb = []
        for i in range(T // 128):
            t = persist.tile([128, HPC * (D + 1)], dt_in, name=f"v_{i}", tag=f"v_{i}")
            for h in range(HPC):
                nc.vector.memset(t[:, (D + 1) * h + D : (D + 1) * (h + 1)], 1.0)
            v_sb.append(t)
        ctx_n = {}
        for b in range(B):
            for h in range(HPC):
                ctx_n[(b, h)] = persist.tile(
                    [D, S], dt_in, name=f"ctx_{b}_{h}", tag=f"ctx_{b}_{h}"
                )

        # ---- stage 1: projections + RoPE ----
        for c in range(NCH):
            c0 = TC * c
            xt = [None] * KE
            for k in range(KE):
                t = xt_pool.tile([128, TC], dt_in, name=f"xt_{c}_{k}", tag="xt")
                nc.sync.dma_start(t[:], xT.ap()[128 * k : 128 * (k + 1), c0 : c0 + TC])
                xt[k] = t

            psq = ps_acc.tile([HD, TC], F32, name="psq", tag="ps_acc")
            psk = ps_acc.tile([HD, TC], F32, name="psk", tag="ps_acc")
            for k in range(KE):
                nc.tensor.matmul(
                    psq[:], mmcast(wq_sb[k][:]), mmcast(xt[k][:]),
                    start=(k == 0), stop=(k == KE - 1),
                )
            for k in range(KE):
                nc.tensor.matmul(
                    psk[:], mmcast(wk_sb[k][:]), mmcast(xt[k][:]),
                    start=(k == 0), stop=(k == KE - 1),
                )
            psv = ps_acc.tile([128, TC], F32, name="psv", tag="ps_acc")
            for j in range(TC // 128):
                for k in range(KE):
                    nc.tensor.matmul(
                        psv[:, 128 * j : 128 * (j + 1)],
                        mmcast(xt[k][:, 128 * j : 128 * (j + 1)]),
                        mmcast(wv_sb[k][:]),
                        start=(k == 0), stop=(k == KE - 1),
                    )
            for j in range(TC // 128):
                vt = v_sb[(c0 + 128 * j) // 128]
                for h in range(HPC):
                    nc.vector.tensor_tensor(
                        vt[:, (D + 1) * h : (D + 1) * h + D],
                        psv[:, 128 * j + D * h : 128 * j + D * (h + 1)],
                        bvb_sb[:, D * h : D * (h + 1)],
                        op=OP.add,
                    )
            for nm, ps, b_sb, out in (
                ("q", psq, bq_sb, q_rope),
                ("k", psk, bk_sb, k_rope),
            ):
                raw = qkraw_pool.tile([HD, TC], dt_in, name=f"{nm}raw", tag="qkraw")
                nc.vector.tensor_scalar_add(raw[:], ps[:], b_sb[:, 0:1])
                psrot = ps_mmt.tile([HD, TC], F32, name="psrot", tag="ps_mmt")
                nc.tensor.matmul(
                    psrot[:], mmcast(rot_sb[:]), mmcast(raw[:]), start=True, stop=True
                )
                sprod = rope_tmp.tile([HD, TC], F32, name="sprod", tag="ropetmp")
                nc.vector.tensor_tensor(
                    sprod[:], psrot[:], sin_sb[:, c0 : c0 + TC], op=OP.mult
                )
                cprod = rope_tmp.tile([HD, TC], F32, name="cprod", tag="ropetmp")
                nc.vector.tensor_tensor(
                    cprod[:], raw[:], cos_sb[:, c0 : c0 + TC], op=OP.mult
                )
                nc.vector.tensor_tensor(
                    out[:, c0 : c0 + TC], cprod[:], sprod[:], op=OP.add
                )

        # ---- stage 2: attention ----
        for b in range(B):
            t0 = b * S
            for qc in range(NQC):
                q0 = t0 + QC * qc
                psc = [
                    ps_ctx.tile([D + 1, QC], F32, name=f"psctx{h}", tag="ps_ctx")
                    for h in range(HPC)
                ]
                for kt in range(NKT):
                    k0 = t0 + 128 * kt
                    ex = [None] * HPC
                    for h in range(HPC):
                        pss = ps_mmt.tile([128, QC], F32, name="pss", tag="ps_mmt")
                        nc.tensor.matmul(
                            pss[:],
                            mmcast(k_rope[D * h : D * (h + 1), k0 : k0 + 128]),
                            mmcast(q_rope[D * h : D * (h + 1), q0 : q0 + QC]),
                            start=True, stop=True,
                        )
                        ext = exps_pool.tile([128, QC], dt_in, name=f"ex{h}", tag="exps")
                        nc.scalar.activation(ext[:], pss[:], AF.Exp, scale=scale)
                        ex[h] = ext
                    for h in range(HPC):
                        nc.tensor.matmul(
                            psc[h][:],
                            mmcast(v_sb[k0 // 128][:, (D + 1) * h : (D + 1) * (h + 1)]),
                            mmcast(ex[h][:]),
                            start=(kt == 0), stop=(kt == NKT - 1),
                        )
                for h in range(HPC):
                    zr = zr_pool.tile([1, QC], F32, name="zrec", tag="zr")
                    nc.vector.reciprocal(zr[:], psc[h][D : D + 1, :])
                    zd = dram.tile([QC], F32, name="zd", tag="zd")
                    nc.sync.dma_start(zd[:], zr[:])
                    zb = zb_pool.tile([D, QC], F32, name="zb", tag="zb")
                    zd_ap = zd[:]
                    nc.sync.dma_start(
                        zb[:],
                        bass.AP(
                            tensor=zd_ap.tensor, offset=zd_ap.offset,
                            ap=[[0, D], [1, QC]],
                        ),
                    )
                    nc.vector.tensor_tensor(
                        ctx_n[(b, h)][:, QC * qc : QC * (qc + 1)],
                        psc[h][0:D, :], zb[:], op=OP.mult,
                    )

        # ---- stage 3: output projection (partial: this core's Wo rows) ----
        for b in range(B):
            t0 = b * S
            for j in range(S // 128):
                for e in range(E // 512):
                    pso = ps_mmt.tile([128, 512], F32, name="pso", tag="ps_mmt")
                    for h in range(HPC):
                        nc.tensor.matmul(
                            pso[:],
                            mmcast(ctx_n[(b, h)][:, 128 * j : 128 * (j + 1)]),
                            mmcast(wo_sb[h][:, 512 * e : 512 * (e + 1)]),
                            start=(h == 0), stop=(h == HPC - 1),
                        )
                    osb = osb_pool.tile([128, 512], F32, name="osb", tag="osb")
                    nc.vector.tensor_copy(osb[:], pso[:])
                    nc.sync.dma_start(
                        yp.ap()[t0 + 128 * j : t0 + 128 * (j + 1), 512 * e : 512 * (e + 1)],
                        osb[:],
                    )

    nc.compile()
    return nc


def _rope_tables():
    inv_freq = 1.0 / (10000.0 ** (np.arange(0, D, 2, dtype=np.float32) / D))
    t = np.arange(S, dtype=np.float32)
    freqs = np.outer(t, inv_freq).astype(np.float32)
    emb = np.concatenate([freqs, freqs], axis=-1)
    return np.cos(emb).astype(np.float32), np.sin(emb).astype(np.float32)


def _rot_matrix():
    R = np.zeros((HD, HD), np.float32)
    for hh in range(HPC):
        for do in range(D):
            po = D * hh + do
            if do < D // 2:
                R[D * hh + do + D // 2, po] = -1.0
            else:
                R[D * hh + do - D // 2, po] = 1.0
    return R


def kernel(x, Wq, bq, Wk, bk, Wv, bv, Wo, bo):
    global LAST_RESULTS
    import ml_dtypes

    x = np.asarray(x, dtype=np.float32)
    Wq, bq = np.asarray(Wq, np.float32), np.asarray(bq, np.float32)
    Wk, bk = np.asarray(Wk, np.float32), np.asarray(bk, np.float32)
    Wv, bv = np.asarray(Wv, np.float32), np.asarray(bv, np.float32)
    Wo, bo = np.asarray(Wo, np.float32), np.asarray(bo, np.float32)

    mode = MM_MODE
    dt_np = ml_dtypes.bfloat16 if mode == "bf16" else np.float32
    T = B * S

    if mode not in _NC_CACHE:
        _NC_CACHE[mode] = build_mha_nc(mode)
    nc = _NC_CACHE[mode]

    xT = np.ascontiguousarray(x.reshape(T, E).T).astype(dt_np)
    cos, sin = _rope_tables()
    cosT = np.tile(np.ascontiguousarray(cos.T), (HPC, B)).astype(np.float32)
    sinT = np.tile(np.ascontiguousarray(sin.T), (HPC, B)).astype(np.float32)
    R = _rot_matrix().astype(dt_np)

    in_maps = []
    for c in range(N_CORES):
        sl = slice(HD * c, HD * (c + 1))
        in_maps.append(
            {
                "xT": xT,
                "wq": np.ascontiguousarray(Wq[:, sl]).astype(dt_np),
                "wk": np.ascontiguousarray(Wk[:, sl]).astype(dt_np),
                "wv": np.ascontiguousarray(Wv[:, sl]).astype(dt_np),
                "bq": np.ascontiguousarray(bq[sl][:, None]).astype(np.float32),
                "bk": np.ascontiguousarray(bk[sl][:, None]).astype(np.float32),
                "bv": np.ascontiguousarray(bv[sl][None, :]).astype(np.float32),
                "wo0": np.ascontiguousarray(Wo[HD * c : HD * c + D, :]).astype(dt_np),
                "wo1": np.ascontiguousarray(Wo[HD * c + D : HD * (c + 1), :]).astype(dt_np),
                "cosT": cosT,
                "sinT": sinT,
                "rot": R,
            }
        )

    res = bass_utils.run_bass_kernel_spmd(nc, in_maps, core_ids=list(range(N_CORES)))
    LAST_RESULTS = res

    out = np.zeros((T, E), np.float64)
    for c in range(N_CORES):
        out += res.results[c]["yp"].astype(np.float64)
    out += bo.astype(np.float64)
    return out.astype(np.float32).reshape(B, S, E)


# revision 3
# speedup vs baseline: 1.4646x; 1.4646x over previous
"""Trainium2 Bass kernel for nn_MultiHeadAttention (RoPE MHA, B=2 S=2048 E=1024 H=16).

Sharding: tensor-parallel over heads — 2 heads per core on 8 cores. Each core
computes its heads' q/k/v projections, RoPE, attention, and the partial output
projection (its rows of Wo); the host sums the 8 partials and adds bo.

Per-core device program layouts:
  q/k as [d, token] (transposed) so attention scores come out as [ks, qs] and
  softmax's row-sum is obtained by appending a ones column to v (the Z row
  falls out of the same matmul that computes ctx). Normalization multiplies
  by 1/Z broadcast across partitions via a DRAM bounce. RoPE's rotate_half is
  a signed-permutation matmul on the tensor engine.
"""

import os
import sys
from contextlib import ExitStack

import numpy as np

for _p in ("/opt/trn_rl_repo", "/opt/pypackages"):
    if _p not in sys.path and os.path.isdir(_p):
        sys.path.append(_p)

import concourse.bass as bass
import concourse.mybir as mybir
import concourse.tile as tile
from concourse import bacc
from concourse import bass_utils

F32 = mybir.dt.float32
AF = mybir.ActivationFunctionType
OP = mybir.AluOpType

B = 2
S = 2048
E = 1024
H = 16
D = 64
N_CORES = 8
HPC = H // N_CORES  # heads per core = 2
HD = HPC * D  # 128

MM_MODE = os.environ.get("MHA_MM_MODE", "f32r")  # 'f32' | 'f32r' | 'bf16'

LAST_RESULTS = None  # BassKernelResults of the most recent run (for test harness)
_NC_CACHE = {}


def build_mha_nc(mm_mode):
    T = B * S
    TC = 512  # token chunk for projections
    NCH = T // TC
    QC = 512  # query chunk in attention
    NQC = S // QC
    NKT = S // 128  # key tiles per batch
    KE = E // 128  # contraction tiles for projections

    dt_in = {"bf16": mybir.dt.bfloat16, "f32r": mybir.dt.float32r, "f32": F32}[mm_mode]

    def mmcast(ap):
        return ap

    nc = bacc.Bacc(None, target_bir_lowering=False, debug=False)

    xT = nc.dram_tensor("xT", [E, T], dt_in, kind="ExternalInput")
    wq = nc.dram_tensor("wq", [E, HD], dt_in, kind="ExternalInput")
    wk = nc.dram_tensor("wk", [E, HD], dt_in, kind="ExternalInput")
    wv = nc.dram_tensor("wv", [E, HD], dt_in, kind="ExternalInput")
    bq = nc.dram_tensor("bq", [HD, 1], F32, kind="ExternalInput")
    bk = nc.dram_tensor("bk", [HD, 1], F32, kind="ExternalInput")
    bv = nc.dram_tensor("bv", [1, HD], F32, kind="ExternalInput")
    wo0 = nc.dram_tensor("wo0", [D, E], dt_in, kind="ExternalInput")
    wo1 = nc.dram_tensor("wo1", [D, E], dt_in, kind="ExternalInput")
    cosT = nc.dram_tensor("cosT", [HD, T], F32, kind="ExternalInput")
    sinT = nc.dram_tensor("sinT", [HD, T], F32, kind="ExternalInput")
    rot = nc.dram_tensor("rot", [HD, HD], dt_in, kind="ExternalInput")
    ones = nc.dram_tensor("ones", [1, 1], dt_in, kind="ExternalInput")
    yp = nc.dram_tensor("yp", [T, E], F32, kind="ExternalOutput")

    scale = 1.0 / np.sqrt(D)

    with tile.TileContext(nc) as tc, ExitStack() as ctx:
        const = ctx.enter_context(tc.tile_pool(name="const", bufs=1))
        xt_pool = ctx.enter_context(tc.tile_pool(name="xt", bufs=KE + 2))
        qkraw_pool = ctx.enter_context(tc.tile_pool(name="qkraw", bufs=3))
        rope_tmp = ctx.enter_context(tc.tile_pool(name="ropetmp", bufs=3))
        persist = ctx.enter_context(tc.tile_pool(name="persist", bufs=1))
        exps_pool = ctx.enter_context(tc.tile_pool(name="exps", bufs=4))
        zr_pool = ctx.enter_context(tc.tile_pool(name="zr", bufs=4))
        zb_pool = ctx.enter_context(tc.tile_pool(name="zb", bufs=4))
        osb_pool = ctx.enter_context(tc.tile_pool(name="osb", bufs=4))
        dram = ctx.enter_context(tc.tile_pool(name="dram", bufs=6, space="DRAM"))

        ps_acc = ctx.enter_context(tc.tile_pool(name="ps_acc", bufs=3, space="PSUM"))
        ps_mmt = ctx.enter_context(tc.tile_pool(name="ps_mmt", bufs=2, space="PSUM"))
        ps_ctx = ctx.enter_context(tc.tile_pool(name="ps_ctx", bufs=2, space="PSUM"))

        def load_const(name, dram_t, shape, dt):
            t = const.tile(shape, dt, name=name, tag=name)
            nc.sync.dma_start(t[:], dram_t.ap())
            return t

        wq_sb = [None] * KE
        wk_sb = [None] * KE
        wv_sb = [None] * KE
        for k in range(KE):
            for nm, dr, arr in (("wq", wq, wq_sb), ("wk", wk, wk_sb), ("wv", wv, wv_sb)):
                t = const.tile([128, HD], dt_in, name=f"{nm}_{k}", tag=f"{nm}_{k}")
                nc.sync.dma_start(t[:], dr.ap()[128 * k : 128 * (k + 1), :])
                arr[k] = t
        bq_sb = load_const("bq_sb", bq, [HD, 1], F32)
        bk_sb = load_const("bk_sb", bk, [HD, 1], F32)
        wo_sb = [
            load_const("wo0_sb", wo0, [D, E], dt_in),
            load_const("wo1_sb", wo1, [D, E], dt_in),
        ]
        rot_sb = load_const("rot_sb", rot, [HD, HD], dt_in)
        cos_sb = load_const("cos_sb", cosT, [HD, T], F32)
        sin_sb = load_const("sin_sb", sinT, [HD, T], F32)
        bvb_sb = const.tile([128, HD], F32, name="bvb_sb", tag="bvb_sb")
        nc.sync.dma_start(
            bvb_sb[:],
            bass.AP(tensor=bv.ap().tensor, offset=bv.ap().offset, ap=[[0, 128], [1, HD]]),
        )

        q_rope = persist.tile([HD, T], dt_in, name="q_rope", tag="q_rope")
        k_rope = persist.tile([HD, T], dt_in, name="k_rope", tag="k_rope")
        v_sb = []
        for i in range(T // 128):
            t = persist.tile([128, HPC * (D + 1)], dt_in, name=f"v_{i}", tag=f"v_{i}")
            ones_ap = ones.ap()
            for h in range(HPC):
                nc.sync.dma_start(
                    t[:, (D + 1) * h + D : (D + 1) * (h + 1)],
                    bass.AP(tensor=ones_ap.tensor, offset=ones_ap.offset, ap=[[0, 128], [1, 1]]),
                )
            v_sb.append(t)
        ctx_n = {}
        for b in range(B):
            for h in range(HPC):
                ctx_n[(b, h)] = persist.tile(
                    [D, S], dt_in, name=f"ctx_{b}_{h}", tag=f"ctx_{b}_{h}"
                )

        # ---- stage 1: projections + RoPE ----
        for c in range(NCH):
            c0 = TC * c
            xt = [None] * KE
            for k in range(KE):
                t = xt_pool.tile([128, TC], dt_in, name=f"xt_{c}_{k}", tag="xt")
                nc.sync.dma_start(t[:], xT.ap()[128 * k : 128 * (k + 1), c0 : c0 + TC])
                xt[k] = t

            psq = ps_acc.tile([HD, TC], F32, name="psq", tag="ps_acc")
            psk = ps_acc.tile([HD, TC], F32, name="psk", tag="ps_acc")
            for k in range(KE):
                nc.tensor.matmul(
                    psq[:], mmcast(wq_sb[k][:]), mmcast(xt[k][:]),
                    start=(k == 0), stop=(k == KE - 1),
                )
            for k in range(KE):
                nc.tensor.matmul(
                    psk[:], mmcast(wk_sb[k][:]), mmcast(xt[k][:]),
                    start=(k == 0), stop=(k == KE - 1),
                )
            psv = ps_acc.tile([128, TC], F32, name="psv", tag="ps_acc")
            for j in range(TC // 128):
                for k in range(KE):
                    nc.tensor.matmul(
                        psv[:, 128 * j : 128 * (j + 1)],
                        mmcast(xt[k][:, 128 * j : 128 * (j + 1)]),
                        mmcast(wv_sb[k][:]),
                        start=(k == 0), stop=(k == KE - 1),
                    )
            for j in range(TC // 128):
                vt = v_sb[(c0 + 128 * j) // 128]
                for h in range(HPC):
                    nc.vector.tensor_tensor(
                        vt[:, (D + 1) * h : (D + 1) * h + D],
                        psv[:, 128 * j + D * h : 128 * j + D * (h + 1)],
                        bvb_sb[:, D * h : D * (h + 1)],
                        op=OP.add,
                    )
            for nm, ps, b_sb, out in (
                ("q", psq, bq_sb, q_rope),
                ("k", psk, bk_sb, k_rope),
            ):
                raw = qkraw_pool.tile([HD, TC], dt_in, name=f"{nm}raw", tag="qkraw")
                nc.vector.tensor_scalar_add(raw[:], ps[:], b_sb[:, 0:1])
                psrot = ps_mmt.tile([HD, TC], F32, name="psrot", tag="ps_mmt")
                nc.tensor.matmul(
                    psrot[:], mmcast(rot_sb[:]), mmcast(raw[:]), start=True, stop=True
                )
                sprod = rope_tmp.tile([HD, TC], F32, name="sprod", tag="ropetmp")
                nc.vector.tensor_tensor(
                    sprod[:], psrot[:], sin_sb[:, c0 : c0 + TC], op=OP.mult
                )
                cprod = rope_tmp.tile([HD, TC], F32, name="cprod", tag="ropetmp")
                nc.vector.tensor_tensor(
                    cprod[:], raw[:], cos_sb[:, c0 : c0 + TC], op=OP.mult
                )
                nc.vector.tensor_tensor(
                    out[:, c0 : c0 + TC], cprod[:], sprod[:], op=OP.add
                )

        # ---- stage 2: attention ----
        for b in range(B):
            t0 = b * S
            for qc in range(NQC):
                q0 = t0 + QC * qc
                psc = [
                    ps_ctx.tile([D + 1, QC], F32, name=f"psctx{h}", tag="ps_ctx")
                    for h in range(HPC)
                ]
                for kt in range(NKT):
                    k0 = t0 + 128 * kt
                    ex = [None] * HPC
                    for h in range(HPC):
                        pss = ps_mmt.tile([128, QC], F32, name="pss", tag="ps_mmt")
                        nc.tensor.matmul(
                            pss[:],
                            mmcast(k_rope[D * h : D * (h + 1), k0 : k0 + 128]),
                            mmcast(q_rope[D * h : D * (h + 1), q0 : q0 + QC]),
                            start=True, stop=True,
                        )
                        ext = exps_pool.tile([128, QC], dt_in, name=f"ex{h}", tag="exps")
                        nc.scalar.activation(ext[:], pss[:], AF.Exp, scale=scale)
                        ex[h] = ext
                    for h in range(HPC):
                        nc.tensor.matmul(
                            psc[h][:],
                            mmcast(v_sb[k0 // 128][:, (D + 1) * h : (D + 1) * (h + 1)]),
                            mmcast(ex[h][:]),
                            start=(kt == 0), stop=(kt == NKT - 1),
                        )
                for h in range(HPC):
                    zr = zr_pool.tile([1, QC], F32, name="zrec", tag="zr")
                    nc.vector.reciprocal(zr[:], psc[h][D : D + 1, :])
                    zd = dram.tile([QC], F32, name="zd", tag="zd")
                    nc.sync.dma_start(zd[:], zr[:])
                    zb = zb_pool.tile([D, QC], F32, name="zb", tag="zb")
                    zd_ap = zd[:]
                    nc.sync.dma_start(
                        zb[:],
                        bass.AP(
                            tensor=zd_ap.tensor, offset=zd_ap.offset,
                            ap=[[0, D], [1, QC]],
                        ),
                    )
                    nc.vector.tensor_tensor(
                        ctx_n[(b, h)][:, QC * qc : QC * (qc + 1)],
                        psc[h][0:D, :], zb[:], op=OP.mult,
                    )

        # ---- stage 3: output projection (partial: this core's Wo rows) ----
        for b in range(B):
            t0 = b * S
            for j in range(S // 128):
                for e in range(E // 512):
                    pso = ps_mmt.tile([128, 512], F32, name="pso", tag="ps_mmt")
                    for h in range(HPC):
                        nc.tensor.matmul(
                            pso[:],
                            mmcast(ctx_n[(b, h)][:, 128 * j : 128 * (j + 1)]),
                            mmcast(wo_sb[h][:, 512 * e : 512 * (e + 1)]),
                            start=(h == 0), stop=(h == HPC - 1),
                        )
                    osb = osb_pool.tile([128, 512], F32, name="osb", tag="osb")
                    nc.vector.tensor_copy(osb[:], pso[:])
                    nc.sync.dma_start(
                        yp.ap()[t0 + 128 * j : t0 + 128 * (j + 1), 512 * e : 512 * (e + 1)],
                        osb[:],
                    )

    nc.compile()
    return nc


def _rope_tables():
    inv_freq = 1.0 / (10000.0 ** (np.arange(0, D, 2, dtype=np.float32) / D))
    t = np.arange(S, dtype=np.float32)
    freqs = np.outer(t, inv_freq).astype(np.float32)
    emb = np.concatenate([freqs, freqs], axis=-1)
    return np.cos(emb).astype(np.float32), np.sin(emb).astype(np.float32)


def _rot_matrix():
    R = np.zeros((HD, HD), np.float32)
    for hh in range(HPC):
        for do in range(D):
            po = D * hh + do
            if do < D // 2:
                R[D * hh + do + D // 2, po] = -1.0
            else:
                R[D * hh + do - D // 2, po] = 1.0
    return R


def kernel(x, Wq, bq, Wk, bk, Wv, bv, Wo, bo):
    global LAST_RESULTS
    import ml_dtypes

    x = np.asarray(x, dtype=np.float32)
    Wq, bq = np.asarray(Wq, np.float32), np.asarray(bq, np.float32)
    Wk, bk = np.asarray(Wk, np.float32), np.asarray(bk, np.float32)
    Wv, bv = np.asarray(Wv, np.float32), np.asarray(bv, np.float32)
    Wo, bo = np.asarray(Wo, np.float32), np.asarray(bo, np.float32)

    mode = MM_MODE
    dt_np = ml_dtypes.bfloat16 if mode == "bf16" else np.float32
    T = B * S

    if mode not in _NC_CACHE:
        _NC_CACHE[mode] = build_mha_nc(mode)
    nc = _NC_CACHE[mode]

    xT = np.ascontiguousarray(x.reshape(T, E).T).astype(dt_np)
    cos, sin = _rope_tables()
    cosT = np.tile(np.ascontiguousarray(cos.T), (HPC, B)).astype(np.float32)
    sinT = np.tile(np.ascontiguousarray(sin.T), (HPC, B)).astype(np.float32)
    R = _rot_matrix().astype(dt_np)

    in_maps = []
    for c in range(N_CORES):
        sl = slice(HD * c, HD * (c + 1))
        in_maps.append(
            {
                "xT": xT,
                "wq": np.ascontiguousarray(Wq[:, sl]).astype(dt_np),
                "wk": np.ascontiguousarray(Wk[:, sl]).astype(dt_np),
                "wv": np.ascontiguousarray(Wv[:, sl]).astype(dt_np),
                "bq": np.ascontiguousarray(bq[sl][:, None]).astype(np.float32),
                "bk": np.ascontiguousarray(bk[sl][:, None]).astype(np.float32),
                "bv": np.ascontiguousarray(bv[sl][None, :]).astype(np.float32),
                "wo0": np.ascontiguousarray(Wo[HD * c : HD * c + D, :]).astype(dt_np),
                "wo1": np.ascontiguousarray(Wo[HD * c + D : HD * (c + 1), :]).astype(dt_np),
                "cosT": cosT,
                "sinT": sinT,
                "rot": R,
                "ones": np.ones((1, 1), dt_np),
            }
        )

    res = bass_utils.run_bass_kernel_spmd(nc, in_maps, core_ids=list(range(N_CORES)))
    LAST_RESULTS = res

    out = np.zeros((T, E), np.float64)
    for c in range(N_CORES):
        out += res.results[c]["yp"].astype(np.float64)
    out += bo.astype(np.float64)
    return out.astype(np.float32).reshape(B, S, E)


# revision 5
# speedup vs baseline: 2.1175x; 1.4458x over previous
"""Trainium2 Bass kernel for nn_MultiHeadAttention (RoPE MHA, B=2 S=2048 E=1024 H=16).

Sharding: tensor-parallel over heads — 2 heads per core on 8 cores. Each core
computes its heads' q/k/v projections, RoPE, attention, and the partial output
projection (its rows of Wo); the host sums the 8 partials and adds bo.

Per-core device program layouts:
  q/k as [d, token] (transposed) so attention scores come out as [ks, qs] and
  softmax's row-sum is obtained by appending a ones column to v (the Z row
  falls out of the same matmul that computes ctx). Normalization multiplies
  by 1/Z broadcast across partitions via a DRAM bounce. RoPE's rotate_half is
  a signed-permutation matmul on the tensor engine.
"""

import os
import sys
from contextlib import ExitStack

import numpy as np

for _p in ("/opt/trn_rl_repo", "/opt/pypackages"):
    if _p not in sys.path and os.path.isdir(_p):
        sys.path.append(_p)

import concourse.bass as bass
import concourse.mybir as mybir
import concourse.tile as tile
from concourse import bacc
from concourse import bass_utils

F32 = mybir.dt.float32
AF = mybir.ActivationFunctionType
OP = mybir.AluOpType

B = 2
S = 2048
E = 1024
H = 16
D = 64
N_CORES = 8
HPC = H // N_CORES  # heads per core = 2
HD = HPC * D  # 128

MM_MODE = os.environ.get("MHA_MM_MODE", "f32r")  # 'f32' | 'f32r' | 'bf16'

LAST_RESULTS = None  # BassKernelResults of the most recent run (for test harness)
_NC_CACHE = {}


def build_mha_nc(mm_mode):
    T = B * S
    TC = 512  # token chunk for projections
    NCH = T // TC
    QC = min(512, S)  # query chunk in attention (N<=512: one PSUM bank per matmul)
    NQC = S // QC
    NKT = S // 128  # key tiles per batch
    KE = E // 128  # contraction tiles for projections

    dt_in = {"bf16": mybir.dt.bfloat16, "f32r": mybir.dt.float32r, "f32": F32}[mm_mode]

    def mmcast(ap):
        return ap

    nc = bacc.Bacc(None, target_bir_lowering=False, debug=False)

    xT = nc.dram_tensor("xT", [E, T], dt_in, kind="ExternalInput")
    wq = nc.dram_tensor("wq", [E, HD], dt_in, kind="ExternalInput")
    wk = nc.dram_tensor("wk", [E, HD], dt_in, kind="ExternalInput")
    wv = nc.dram_tensor("wv", [E, HD], dt_in, kind="ExternalInput")
    bq = nc.dram_tensor("bq", [HD, 1], F32, kind="ExternalInput")
    bk = nc.dram_tensor("bk", [HD, 1], F32, kind="ExternalInput")
    bv = nc.dram_tensor("bv", [1, HD], F32, kind="ExternalInput")
    wo0 = nc.dram_tensor("wo0", [D, E], dt_in, kind="ExternalInput")
    wo1 = nc.dram_tensor("wo1", [D, E], dt_in, kind="ExternalInput")
    cosT = nc.dram_tensor("cosT", [HD, T], F32, kind="ExternalInput")
    sinT = nc.dram_tensor("sinT", [HD, T], F32, kind="ExternalInput")
    rot = nc.dram_tensor("rot", [HD, HD], dt_in, kind="ExternalInput")
    ones = nc.dram_tensor("ones", [1, 1], dt_in, kind="ExternalInput")
    yp = nc.dram_tensor("yp", [T, E], F32, kind="ExternalOutput")

    scale = 1.0 / np.sqrt(D)

    with tile.TileContext(nc) as tc, ExitStack() as ctx:
        const = ctx.enter_context(tc.tile_pool(name="const", bufs=1))
        xt_pool = ctx.enter_context(tc.tile_pool(name="xt", bufs=KE + 2))
        qkraw_pool = ctx.enter_context(tc.tile_pool(name="qkraw", bufs=3))
        rope_tmp = ctx.enter_context(tc.tile_pool(name="ropetmp", bufs=3))
        persist = ctx.enter_context(tc.tile_pool(name="persist", bufs=1))
        exps_pool = ctx.enter_context(tc.tile_pool(name="exps", bufs=4))
        zr_pool = ctx.enter_context(tc.tile_pool(name="zr", bufs=4))
        zb_pool = ctx.enter_context(tc.tile_pool(name="zb", bufs=4))
        osb_pool = ctx.enter_context(tc.tile_pool(name="osb", bufs=4))
        dram = ctx.enter_context(tc.tile_pool(name="dram", bufs=6, space="DRAM"))


        def load_const(name, dram_t, shape, dt):
            t = const.tile(shape, dt, name=name, tag=name)
            nc.sync.dma_start(t[:], dram_t.ap())
            return t

        wq_sb = [None] * KE
        wk_sb = [None] * KE
        wv_sb = [None] * KE
        for k in range(KE):
            for nm, dr, arr in (("wq", wq, wq_sb), ("wk", wk, wk_sb), ("wv", wv, wv_sb)):
                t = const.tile([128, HD], dt_in, name=f"{nm}_{k}", tag=f"{nm}_{k}")
                nc.sync.dma_start(t[:], dr.ap()[128 * k : 128 * (k + 1), :])
                arr[k] = t
        bq_sb = load_const("bq_sb", bq, [HD, 1], F32)
        bk_sb = load_const("bk_sb", bk, [HD, 1], F32)
        wo_sb = [
            load_const("wo0_sb", wo0, [D, E], dt_in),
            load_const("wo1_sb", wo1, [D, E], dt_in),
        ]
        rot_sb = load_const("rot_sb", rot, [HD, HD], dt_in)
        cos_sb = load_const("cos_sb", cosT, [HD, T], F32)
        sin_sb = load_const("sin_sb", sinT, [HD, T], F32)
        bvb_sb = const.tile([128, HD], F32, name="bvb_sb", tag="bvb_sb")
        nc.sync.dma_start(
            bvb_sb[:],
            bass.AP(tensor=bv.ap().tensor, offset=bv.ap().offset, ap=[[0, 128], [1, HD]]),
        )

        q_rope = persist.tile([HD, T], dt_in, name="q_rope", tag="q_rope")
        k_rope = persist.tile([HD, T], dt_in, name="k_rope", tag="k_rope")
        v_sb = []
        for i in range(T // 128):
            t = persist.tile([128, HPC * (D + 1)], dt_in, name=f"v_{i}", tag=f"v_{i}")
            ones_ap = ones.ap()
            for h in range(HPC):
                nc.sync.dma_start(
                    t[:, (D + 1) * h + D : (D + 1) * (h + 1)],
                    bass.AP(tensor=ones_ap.tensor, offset=ones_ap.offset, ap=[[0, 128], [1, 1]]),
                )
            v_sb.append(t)
        ctx_n = {}
        for b in range(B):
            for h in range(HPC):
                ctx_n[(b, h)] = persist.tile(
                    [D, S], dt_in, name=f"ctx_{b}_{h}", tag=f"ctx_{b}_{h}"
                )

        # ---- stage 1: projections + RoPE ----
        s1 = ExitStack()
        ps_acc = s1.enter_context(tc.tile_pool(name="ps_acc", bufs=3, space="PSUM"))
        ps_mmt = s1.enter_context(tc.tile_pool(name="ps_mmt", bufs=2, space="PSUM"))
        for c in range(NCH):
            c0 = TC * c
            xt = [None] * KE
            for k in range(KE):
                t = xt_pool.tile([128, TC], dt_in, name=f"xt_{c}_{k}", tag="xt")
                nc.sync.dma_start(t[:], xT.ap()[128 * k : 128 * (k + 1), c0 : c0 + TC])
                xt[k] = t

            psq = ps_acc.tile([HD, TC], F32, name="psq", tag="ps_acc")
            psk = ps_acc.tile([HD, TC], F32, name="psk", tag="ps_acc")
            for k in range(KE):
                nc.tensor.matmul(
                    psq[:], mmcast(wq_sb[k][:]), mmcast(xt[k][:]),
                    start=(k == 0), stop=(k == KE - 1),
                )
            for k in range(KE):
                nc.tensor.matmul(
                    psk[:], mmcast(wk_sb[k][:]), mmcast(xt[k][:]),
                    start=(k == 0), stop=(k == KE - 1),
                )
            psv = ps_acc.tile([128, TC], F32, name="psv", tag="ps_acc")
            for j in range(TC // 128):
                for k in range(KE):
                    nc.tensor.matmul(
                        psv[:, 128 * j : 128 * (j + 1)],
                        mmcast(xt[k][:, 128 * j : 128 * (j + 1)]),
                        mmcast(wv_sb[k][:]),
                        start=(k == 0), stop=(k == KE - 1),
                    )
            for j in range(TC // 128):
                vt = v_sb[(c0 + 128 * j) // 128]
                for h in range(HPC):
                    nc.vector.tensor_tensor(
                        vt[:, (D + 1) * h : (D + 1) * h + D],
                        psv[:, 128 * j + D * h : 128 * j + D * (h + 1)],
                        bvb_sb[:, D * h : D * (h + 1)],
                        op=OP.add,
                    )
            for nm, ps, b_sb, out in (
                ("q", psq, bq_sb, q_rope),
                ("k", psk, bk_sb, k_rope),
            ):
                raw = qkraw_pool.tile([HD, TC], dt_in, name=f"{nm}raw", tag="qkraw")
                nc.vector.tensor_scalar_add(raw[:], ps[:], b_sb[:, 0:1])
                psrot = ps_mmt.tile([HD, TC], F32, name="psrot", tag="ps_mmt")
                nc.tensor.matmul(
                    psrot[:], mmcast(rot_sb[:]), mmcast(raw[:]), start=True, stop=True
                )
                sprod = rope_tmp.tile([HD, TC], F32, name="sprod", tag="ropetmp")
                nc.vector.tensor_tensor(
                    sprod[:], psrot[:], sin_sb[:, c0 : c0 + TC], op=OP.mult
                )
                cprod = rope_tmp.tile([HD, TC], F32, name="cprod", tag="ropetmp")
                nc.vector.tensor_tensor(
                    cprod[:], raw[:], cos_sb[:, c0 : c0 + TC], op=OP.mult
                )
                nc.vector.tensor_tensor(
                    out[:, c0 : c0 + TC], cprod[:], sprod[:], op=OP.add
                )

        s1.close()

        # ---- stage 2: attention ----
        s2 = ExitStack()
        ps_s = s2.enter_context(tc.tile_pool(name="ps_s", bufs=2, space="PSUM"))
        ps_ctx = s2.enter_context(tc.tile_pool(name="ps_ctx", bufs=2, space="PSUM"))
        for b in range(B):
            t0 = b * S
            for qc in range(NQC):
                q0 = t0 + QC * qc
                psc = [
                    ps_ctx.tile([D + 1, QC], F32, name=f"psctx{h}", tag="ps_ctx")
                    for h in range(HPC)
                ]
                for kt in range(NKT):
                    k0 = t0 + 128 * kt
                    # both heads' scores into one 2-bank psum tile -> single exp
                    pss = ps_s.tile([128, HPC * QC], F32, name="pss", tag="ps_s")
                    for h in range(HPC):
                        nc.tensor.matmul(
                            pss[:, QC * h : QC * (h + 1)],
                            mmcast(k_rope[D * h : D * (h + 1), k0 : k0 + 128]),
                            mmcast(q_rope[D * h : D * (h + 1), q0 : q0 + QC]),
                            start=True, stop=True,
                        )
                    ex = exps_pool.tile([128, HPC * QC], dt_in, name="ex", tag="exps")
                    nc.scalar.activation(ex[:], pss[:], AF.Exp, scale=scale)
                    for h in range(HPC):
                        nc.tensor.matmul(
                            psc[h][:],
                            mmcast(v_sb[k0 // 128][:, (D + 1) * h : (D + 1) * (h + 1)]),
                            mmcast(ex[:, QC * h : QC * (h + 1)]),
                            start=(kt == 0), stop=(kt == NKT - 1),
                        )
                for h in range(HPC):
                    zln = zr_pool.tile([1, QC], F32, name="zln", tag="zln")
                    nc.scalar.activation(zln[:], psc[h][D : D + 1, :], AF.Ln)
                    zr = zr_pool.tile([1, QC], F32, name="zrec", tag="zr")
                    nc.scalar.activation(zr[:], zln[:], AF.Exp, scale=-1.0)
                    zd = dram.tile([QC], F32, name="zd", tag="zd")
                    nc.sync.dma_start(zd[:], zr[:])
                    zb = zb_pool.tile([D, QC], F32, name="zb", tag="zb")
                    zd_ap = zd[:]
                    nc.sync.dma_start(
                        zb[:],
                        bass.AP(
                            tensor=zd_ap.tensor, offset=zd_ap.offset,
                            ap=[[0, D], [1, QC]],
                        ),
                    )
                    nc.vector.tensor_tensor(
                        ctx_n[(b, h)][:, QC * qc : QC * (qc + 1)],
                        psc[h][0:D, :], zb[:], op=OP.mult,
                    )

        s2.close()

        # ---- stage 3: output projection (partial: this core's Wo rows) ----
        ps_o = ctx.enter_context(tc.tile_pool(name="ps_o", bufs=3, space="PSUM"))
        for b in range(B):
            t0 = b * S
            for j in range(S // 128):
                for e in range(E // 512):
                    pso = ps_o.tile([128, 512], F32, name="pso", tag="ps_o")
                    for h in range(HPC):
                        nc.tensor.matmul(
                            pso[:],
                            mmcast(ctx_n[(b, h)][:, 128 * j : 128 * (j + 1)]),
                            mmcast(wo_sb[h][:, 512 * e : 512 * (e + 1)]),
                            start=(h == 0), stop=(h == HPC - 1),
                        )
                    osb = osb_pool.tile([128, 512], F32, name="osb", tag="osb")
                    nc.vector.tensor_copy(osb[:], pso[:])
                    nc.sync.dma_start(
                        yp.ap()[t0 + 128 * j : t0 + 128 * (j + 1), 512 * e : 512 * (e + 1)],
                        osb[:],
                    )

    nc.compile()
    return nc


def _rope_tables():
    inv_freq = 1.0 / (10000.0 ** (np.arange(0, D, 2, dtype=np.float32) / D))
    t = np.arange(S, dtype=np.float32)
    freqs = np.outer(t, inv_freq).astype(np.float32)
    emb = np.concatenate([freqs, freqs], axis=-1)
    return np.cos(emb).astype(np.float32), np.sin(emb).astype(np.float32)


def _rot_matrix():
    R = np.zeros((HD, HD), np.float32)
    for hh in range(HPC):
        for do in range(D):
            po = D * hh + do
            if do < D // 2:
                R[D * hh + do + D // 2, po] = -1.0
            else:
                R[D * hh + do - D // 2, po] = 1.0
    return R


def kernel(x, Wq, bq, Wk, bk, Wv, bv, Wo, bo):
    global LAST_RESULTS
    import ml_dtypes

    x = np.asarray(x, dtype=np.float32)
    Wq, bq = np.asarray(Wq, np.float32), np.asarray(bq, np.float32)
    Wk, bk = np.asarray(Wk, np.float32), np.asarray(bk, np.float32)
    Wv, bv = np.asarray(Wv, np.float32), np.asarray(bv, np.float32)
    Wo, bo = np.asarray(Wo, np.float32), np.asarray(bo, np.float32)

    mode = MM_MODE
    dt_np = ml_dtypes.bfloat16 if mode == "bf16" else np.float32
    T = B * S

    if mode not in _NC_CACHE:
        _NC_CACHE[mode] = build_mha_nc(mode)
    nc = _NC_CACHE[mode]

    xT = np.ascontiguousarray(x.reshape(T, E).T).astype(dt_np)
    cos, sin = _rope_tables()
    cosT = np.tile(np.ascontiguousarray(cos.T), (HPC, B)).astype(np.float32)
    sinT = np.tile(np.ascontiguousarray(sin.T), (HPC, B)).astype(np.float32)
    R = _rot_matrix().astype(dt_np)

    in_maps = []
    for c in range(N_CORES):
        sl = slice(HD * c, HD * (c + 1))
        in_maps.append(
            {
                "xT": xT,
                "wq": np.ascontiguousarray(Wq[:, sl]).astype(dt_np),
                "wk": np.ascontiguousarray(Wk[:, sl]).astype(dt_np),
                "wv": np.ascontiguousarray(Wv[:, sl]).astype(dt_np),
                "bq": np.ascontiguousarray(bq[sl][:, None]).astype(np.float32),
                "bk": np.ascontiguousarray(bk[sl][:, None]).astype(np.float32),
                "bv": np.ascontiguousarray(bv[sl][None, :]).astype(np.float32),
                "wo0": np.ascontiguousarray(Wo[HD * c : HD * c + D, :]).astype(dt_np),
                "wo1": np.ascontiguousarray(Wo[HD * c + D : HD * (c + 1), :]).astype(dt_np),
                "cosT": cosT,
                "sinT": sinT,
                "rot": R,
                "ones": np.ones((1, 1), dt_np),
            }
        )

    res = bass_utils.run_bass_kernel_spmd(nc, in_maps, core_ids=list(range(N_CORES)))
    LAST_RESULTS = res

    out = np.zeros((T, E), np.float64)
    for c in range(N_CORES):
        out += res.results[c]["yp"].astype(np.float64)
    out += bo.astype(np.float64)
    return out.astype(np.float32).reshape(B, S, E)


# revision 7
# speedup vs baseline: 2.1436x; 1.0123x over previous
"""Trainium2 Bass kernel for nn_MultiHeadAttention (RoPE MHA, B=2 S=2048 E=1024 H=16).

Sharding: tensor-parallel over heads — 2 heads per core on 8 cores. Each core
computes its heads' q/k/v projections, RoPE, attention, and the partial output
projection (its rows of Wo); the host sums the 8 partials and adds bo.

Per-core device program layouts:
  q/k as [d, token] (transposed) so attention scores come out as [ks, qs] and
  softmax's row-sum is obtained by appending a ones column to v (the Z row
  falls out of the same matmul that computes ctx). Normalization multiplies
  by 1/Z broadcast across partitions via a DRAM bounce. RoPE's rotate_half is
  a signed-permutation matmul on the tensor engine.
"""

import os
import sys
from contextlib import ExitStack

import numpy as np

for _p in ("/opt/trn_rl_repo", "/opt/pypackages"):
    if _p not in sys.path and os.path.isdir(_p):
        sys.path.append(_p)

import concourse.bass as bass
import concourse.mybir as mybir
import concourse.tile as tile
from concourse import bacc
from concourse import bass_utils

F32 = mybir.dt.float32
AF = mybir.ActivationFunctionType
OP = mybir.AluOpType

B = 2
S = 2048
E = 1024
H = 16
D = 64
N_CORES = 8
HPC = H // N_CORES  # heads per core = 2
HD = HPC * D  # 128

MM_MODE = os.environ.get("MHA_MM_MODE", "f32r")  # 'f32' | 'f32r' | 'bf16'

LAST_RESULTS = None  # BassKernelResults of the most recent run (for test harness)
_NC_CACHE = {}


def build_mha_nc(mm_mode):
    T = B * S
    TC = 512  # token chunk for projections
    NCH = T // TC
    QC = min(512, S)  # query chunk in attention (N<=512: one PSUM bank per matmul)
    NQC = S // QC
    NKT = S // 128  # key tiles per batch
    KE = E // 128  # contraction tiles for projections

    dt_in = {"bf16": mybir.dt.bfloat16, "f32r": mybir.dt.float32r, "f32": F32}[mm_mode]

    def mmcast(ap):
        return ap

    nc = bacc.Bacc(None, target_bir_lowering=False, debug=False)

    xT = nc.dram_tensor("xT", [E, T], dt_in, kind="ExternalInput")
    wq = nc.dram_tensor("wq", [E, HD], dt_in, kind="ExternalInput")
    wk = nc.dram_tensor("wk", [E, HD], dt_in, kind="ExternalInput")
    wv = nc.dram_tensor("wv", [E, HD], dt_in, kind="ExternalInput")
    bq = nc.dram_tensor("bq", [HD, 1], F32, kind="ExternalInput")
    bk = nc.dram_tensor("bk", [HD, 1], F32, kind="ExternalInput")
    bv = nc.dram_tensor("bv", [HD, 1], F32, kind="ExternalInput")
    wo = nc.dram_tensor("wo", [HD, E], dt_in, kind="ExternalInput")
    cosT = nc.dram_tensor("cosT", [HD, T], F32, kind="ExternalInput")
    sinT = nc.dram_tensor("sinT", [HD, T], F32, kind="ExternalInput")
    rot = nc.dram_tensor("rot", [HD, HD], dt_in, kind="ExternalInput")
    ones = nc.dram_tensor("ones", [1, 1], dt_in, kind="ExternalInput")
    yp = nc.dram_tensor("yp", [T, E], F32, kind="ExternalOutput")

    scale = 1.0 / np.sqrt(D)

    with tile.TileContext(nc) as tc, ExitStack() as ctx:
        const = ctx.enter_context(tc.tile_pool(name="const", bufs=1))
        xt_pool = ctx.enter_context(tc.tile_pool(name="xt", bufs=KE + 2))
        qkraw_pool = ctx.enter_context(tc.tile_pool(name="qkraw", bufs=3))
        rope_tmp = ctx.enter_context(tc.tile_pool(name="ropetmp", bufs=3))
        persist = ctx.enter_context(tc.tile_pool(name="persist", bufs=1))
        exps_pool = ctx.enter_context(tc.tile_pool(name="exps", bufs=6))
        zr_pool = ctx.enter_context(tc.tile_pool(name="zr", bufs=4))
        zb_pool = ctx.enter_context(tc.tile_pool(name="zb", bufs=4))
        osb_pool = ctx.enter_context(tc.tile_pool(name="osb", bufs=4))
        dram = ctx.enter_context(tc.tile_pool(name="dram", bufs=6, space="DRAM"))


        def load_const(name, dram_t, shape, dt):
            t = const.tile(shape, dt, name=name, tag=name)
            nc.sync.dma_start(t[:], dram_t.ap())
            return t

        wq_sb = [None] * KE
        wk_sb = [None] * KE
        wv_sb = [None] * KE
        for k in range(KE):
            for nm, dr, arr in (("wq", wq, wq_sb), ("wk", wk, wk_sb), ("wv", wv, wv_sb)):
                t = const.tile([128, HD], dt_in, name=f"{nm}_{k}", tag=f"{nm}_{k}")
                nc.sync.dma_start(t[:], dr.ap()[128 * k : 128 * (k + 1), :])
                arr[k] = t
        bq_sb = load_const("bq_sb", bq, [HD, 1], F32)
        bk_sb = load_const("bk_sb", bk, [HD, 1], F32)
        wo_sb = load_const("wo_sb", wo, [HD, E], dt_in)
        from concourse.masks import make_identity
        ident = const.tile([128, 128], dt_in, name="ident", tag="ident")
        make_identity(nc, ident)
        rot_sb = load_const("rot_sb", rot, [HD, HD], dt_in)
        cos_sb = load_const("cos_sb", cosT, [HD, T], F32)
        sin_sb = load_const("sin_sb", sinT, [HD, T], F32)
        bv_sb = load_const("bv_sb", bv, [HD, 1], F32)

        q_rope = persist.tile([HD, T], dt_in, name="q_rope", tag="q_rope")
        k_rope = persist.tile([HD, T], dt_in, name="k_rope", tag="k_rope")
        v_sb = []
        for i in range(T // 128):
            t = persist.tile([128, HPC * (D + 1)], dt_in, name=f"v_{i}", tag=f"v_{i}")
            ones_ap = ones.ap()
            for h in range(HPC):
                nc.sync.dma_start(
                    t[:, (D + 1) * h + D : (D + 1) * (h + 1)],
                    bass.AP(tensor=ones_ap.tensor, offset=ones_ap.offset, ap=[[0, 128], [1, 1]]),
                )
            v_sb.append(t)
        ctx_pack = {}
        for b in range(B):
            ctx_pack[b] = persist.tile([HD, S], dt_in, name=f"ctxp_{b}", tag=f"ctxp_{b}")

        # ---- stage 1: projections (qT/kT/vT) + RoPE + v transpose ----
        s1 = ExitStack()
        ps_acc = s1.enter_context(tc.tile_pool(name="ps_acc", bufs=6, space="PSUM"))
        ps_mmt = s1.enter_context(tc.tile_pool(name="ps_mmt", bufs=2, space="PSUM"))
        for c in range(NCH):
            c0 = TC * c
            xt = [None] * KE
            for k in range(KE):
                t = xt_pool.tile([128, TC], dt_in, name=f"xt_{c}_{k}", tag="xt")
                nc.sync.dma_start(t[:], xT.ap()[128 * k : 128 * (k + 1), c0 : c0 + TC])
                xt[k] = t

            psq = ps_acc.tile([HD, TC], F32, name="psq", tag="ps_acc")
            psk = ps_acc.tile([HD, TC], F32, name="psk", tag="ps_acc")
            psv = ps_acc.tile([HD, TC], F32, name="psv", tag="ps_acc")
            for ps, w_sb in ((psq, wq_sb), (psk, wk_sb), (psv, wv_sb)):
                for k in range(KE):
                    nc.tensor.matmul(
                        ps[:], mmcast(w_sb[k][:]), mmcast(xt[k][:]),
                        start=(k == 0), stop=(k == KE - 1),
                    )
            # vT -> v natural via PE transpose; evict with bias into ones-padded layout
            vraw = qkraw_pool.tile([HD, TC], dt_in, name="vraw", tag="qkraw")
            nc.vector.tensor_scalar_add(vraw[:], psv[:], bv_sb[:, 0:1])
            for j in range(TC // 128):
                pvt = ps_mmt.tile([128, 128], dt_in, name="pvt", tag="ps_mmt")
                nc.tensor.transpose(
                    pvt[:], mmcast(vraw[:, 128 * j : 128 * (j + 1)]), mmcast(ident[:])
                )
                vt = v_sb[(c0 + 128 * j) // 128]
                for h in range(HPC):
                    nc.vector.tensor_copy(
                        vt[:, (D + 1) * h : (D + 1) * h + D],
                        pvt[:, D * h : D * (h + 1)],
                    )
            # q/k: bias evict, rotate matmul, rope combine
            for nm, ps, b_sb, out in (
                ("q", psq, bq_sb, q_rope),
                ("k", psk, bk_sb, k_rope),
            ):
                raw = qkraw_pool.tile([HD, TC], dt_in, name=f"{nm}raw", tag="qkraw")
                nc.vector.tensor_scalar_add(raw[:], ps[:], b_sb[:, 0:1])
                psrot = ps_mmt.tile([HD, TC], F32, name="psrot", tag="ps_mmt")
                nc.tensor.matmul(
                    psrot[:], mmcast(rot_sb[:]), mmcast(raw[:]), start=True, stop=True
                )
                sprod = rope_tmp.tile([HD, TC], F32, name="sprod", tag="ropetmp")
                nc.vector.tensor_tensor(
                    sprod[:], psrot[:], sin_sb[:, c0 : c0 + TC], op=OP.mult
                )
                cprod = rope_tmp.tile([HD, TC], F32, name="cprod", tag="ropetmp")
                nc.vector.tensor_tensor(
                    cprod[:], raw[:], cos_sb[:, c0 : c0 + TC], op=OP.mult
                )
                nc.vector.tensor_tensor(
                    out[:, c0 : c0 + TC], cprod[:], sprod[:], op=OP.add
                )
        s1.close()

        # ---- stage 2: attention ----
        s2 = ExitStack()
        ps_s = s2.enter_context(tc.tile_pool(name="ps_s", bufs=2, space="PSUM"))
        ps_ctx = s2.enter_context(tc.tile_pool(name="ps_ctx", bufs=4, space="PSUM"))
        for b in range(B):
            t0 = b * S
            for qc in range(NQC):
                q0 = t0 + QC * qc
                psc = [
                    ps_ctx.tile([D + 1, QC], F32, name=f"psctx{h}", tag="ps_ctx")
                    for h in range(HPC)
                ]
                for kt in range(NKT):
                    k0 = t0 + 128 * kt
                    # both heads' scores into one 2-bank psum tile -> single exp
                    pss = ps_s.tile([128, HPC * QC], F32, name="pss", tag="ps_s")
                    for h in range(HPC):
                        nc.tensor.matmul(
                            pss[:, QC * h : QC * (h + 1)],
                            mmcast(k_rope[D * h : D * (h + 1), k0 : k0 + 128]),
                            mmcast(q_rope[D * h : D * (h + 1), q0 : q0 + QC]),
                            start=True, stop=True,
                        )
                    ex = exps_pool.tile([128, HPC * QC], dt_in, name="ex", tag="exps")
                    nc.scalar.activation(ex[:], pss[:], AF.Exp, scale=scale)
                    for h in range(HPC):
                        nc.tensor.matmul(
                            psc[h][:],
                            mmcast(v_sb[k0 // 128][:, (D + 1) * h : (D + 1) * (h + 1)]),
                            mmcast(ex[:, QC * h : QC * (h + 1)]),
                            start=(kt == 0), stop=(kt == NKT - 1),
                        )
                for h in range(HPC):
                    zln = zr_pool.tile([1, QC], F32, name="zln", tag="zln")
                    nc.scalar.activation(zln[:], psc[h][D : D + 1, :], AF.Ln)
                    zr = zr_pool.tile([1, QC], F32, name="zrec", tag="zr")
                    nc.scalar.activation(zr[:], zln[:], AF.Exp, scale=-1.0)
                    zd = dram.tile([QC], F32, name="zd", tag="zd")
                    nc.sync.dma_start(zd[:], zr[:])
                    zb = zb_pool.tile([D, QC], F32, name="zb", tag="zb")
                    zd_ap = zd[:]
                    nc.sync.dma_start(
                        zb[:],
                        bass.AP(
                            tensor=zd_ap.tensor, offset=zd_ap.offset,
                            ap=[[0, D], [1, QC]],
                        ),
                    )
                    if h == 0:
                        nc.vector.tensor_tensor(
                            ctx_pack[b][0:D, QC * qc : QC * (qc + 1)],
                            psc[h][0:D, :], zb[:], op=OP.mult,
                        )
                    else:
                        csh = osb_pool.tile([D, QC], dt_in, name="csh", tag="csh")
                        nc.vector.tensor_tensor(csh[:], psc[h][0:D, :], zb[:], op=OP.mult)
                        nc.sync.dma_start(
                            ctx_pack[b][D : 2 * D, QC * qc : QC * (qc + 1)], csh[:]
                        )

        s2.close()

        # ---- stage 3: output projection (partial: this core's Wo rows) ----
        ps_o = ctx.enter_context(tc.tile_pool(name="ps_o", bufs=3, space="PSUM"))
        for b in range(B):
            t0 = b * S
            for j in range(S // 128):
                for e in range(E // 512):
                    pso = ps_o.tile([128, 512], F32, name="pso", tag="ps_o")
                    nc.tensor.matmul(
                        pso[:],
                        mmcast(ctx_pack[b][:, 128 * j : 128 * (j + 1)]),
                        mmcast(wo_sb[:, 512 * e : 512 * (e + 1)]),
                        start=True, stop=True,
                    )
                    osb = osb_pool.tile([128, 512], F32, name="osb", tag="osb")
                    nc.vector.tensor_copy(osb[:], pso[:])
                    nc.sync.dma_start(
                        yp.ap()[t0 + 128 * j : t0 + 128 * (j + 1), 512 * e : 512 * (e + 1)],
                        osb[:],
                    )

    nc.compile()
    return nc


def _rope_tables():
    inv_freq = 1.0 / (10000.0 ** (np.arange(0, D, 2, dtype=np.float32) / D))
    t = np.arange(S, dtype=np.float32)
    freqs = np.outer(t, inv_freq).astype(np.float32)
    emb = np.concatenate([freqs, freqs], axis=-1)
    return np.cos(emb).astype(np.float32), np.sin(emb).astype(np.float32)


def _rot_matrix():
    R = np.zeros((HD, HD), np.float32)
    for hh in range(HPC):
        for do in range(D):
            po = D * hh + do
            if do < D // 2:
                R[D * hh + do + D // 2, po] = -1.0
            else:
                R[D * hh + do - D // 2, po] = 1.0
    return R


def kernel(x, Wq, bq, Wk, bk, Wv, bv, Wo, bo):
    global LAST_RESULTS
    import ml_dtypes

    x = np.asarray(x, dtype=np.float32)
    Wq, bq = np.asarray(Wq, np.float32), np.asarray(bq, np.float32)
    Wk, bk = np.asarray(Wk, np.float32), np.asarray(bk, np.float32)
    Wv, bv = np.asarray(Wv, np.float32), np.asarray(bv, np.float32)
    Wo, bo = np.asarray(Wo, np.float32), np.asarray(bo, np.float32)

    mode = MM_MODE
    dt_np = ml_dtypes.bfloat16 if mode == "bf16" else np.float32
    T = B * S

    if mode not in _NC_CACHE:
        _NC_CACHE[mode] = build_mha_nc(mode)
    nc = _NC_CACHE[mode]

    xT = np.ascontiguousarray(x.reshape(T, E).T).astype(dt_np)
    cos, sin = _rope_tables()
    cosT = np.tile(np.ascontiguousarray(cos.T), (HPC, B)).astype(np.float32)
    sinT = np.tile(np.ascontiguousarray(sin.T), (HPC, B)).astype(np.float32)
    R = _rot_matrix().astype(dt_np)

    in_maps = []
    for c in range(N_CORES):
        sl = slice(HD * c, HD * (c + 1))
        in_maps.append(
            {
                "xT": xT,
                "wq": np.ascontiguousarray(Wq[:, sl]).astype(dt_np),
                "wk": np.ascontiguousarray(Wk[:, sl]).astype(dt_np),
                "wv": np.ascontiguousarray(Wv[:, sl]).astype(dt_np),
                "bq": np.ascontiguousarray(bq[sl][:, None]).astype(np.float32),
                "bk": np.ascontiguousarray(bk[sl][:, None]).astype(np.float32),
                "bv": np.ascontiguousarray(bv[sl][:, None]).astype(np.float32),
                "wo": np.ascontiguousarray(Wo[sl, :]).astype(dt_np),
                "cosT": cosT,
                "sinT": sinT,
                "rot": R,
                "ones": np.ones((1, 1), dt_np),
            }
        )

    res = bass_utils.run_bass_kernel_spmd(nc, in_maps, core_ids=list(range(N_CORES)))
    LAST_RESULTS = res

    out = np.zeros((T, E), np.float64)
    for c in range(N_CORES):
        out += res.results[c]["yp"].astype(np.float64)
    out += bo.astype(np.float64)
    return out.astype(np.float32).reshape(B, S, E)


# revision 9
# speedup vs baseline: 2.4026x; 1.1208x over previous
"""Trainium2 Bass kernel for nn_MultiHeadAttention (RoPE MHA, B=2 S=2048 E=1024 H=16).

Sharding: tensor-parallel over heads — 2 heads per core on 8 cores. Each core
computes its heads' q/k/v projections, RoPE, attention, and the partial output
projection (its rows of Wo); the host sums the 8 partials and adds bo.

Device layouts: q/k as [d, token] (transposed) so attention scores come out as
[ks, qs]; softmax's row-sum falls out of the same matmul that computes ctx via
a ones column appended to v. Normalization multiplies by 1/Z broadcast across
partitions via a DRAM bounce. rotate_half is a signed-permutation matmul.
v is projected transposed (full-width matmuls) and PE-transposed to natural.

Engine balance: exp on ACT (the stage-2 pacer), projection evictions on ACT,
RoPE products + normalize on DVE, RoPE final add on GpSimd, DMAs spread over
sync/scalar/vector/gpsimd queues. Two cross-stage PSUM pools (no stage
barriers): A = 2x2-bank slots (psq/psk/pss), B = 4x1-bank slots (rest).
"""

import os
import sys
from contextlib import ExitStack

import numpy as np

for _p in ("/opt/trn_rl_repo", "/opt/pypackages"):
    if _p not in sys.path and os.path.isdir(_p):
        sys.path.append(_p)

import concourse.bass as bass
import concourse.mybir as mybir
import concourse.tile as tile
from concourse import bacc
from concourse import bass_utils
from concourse.masks import make_identity

F32 = mybir.dt.float32
AF = mybir.ActivationFunctionType
OP = mybir.AluOpType

B = 2
S = 2048
E = 1024
H = 16
D = 64
N_CORES = 8
HPC = H // N_CORES  # heads per core = 2
HD = HPC * D  # 128

MM_MODE = os.environ.get("MHA_MM_MODE", "bf16")  # 'f32' | 'f32r' | 'bf16'

LAST_RESULTS = None  # BassKernelResults of the most recent run (for test harness)
_NC_CACHE = {}


def build_mha_nc(mm_mode):
    T = B * S
    TC = 512  # token chunk for projections
    NCH = T // TC
    QC = min(512, S)  # query chunk in attention (N<=512: one PSUM bank per matmul)
    NQC = S // QC
    NKT = S // 128  # key tiles per batch
    KE = E // 128  # contraction tiles for projections

    dt_in = {"bf16": mybir.dt.bfloat16, "f32r": mybir.dt.float32r, "f32": F32}[mm_mode]

    nc = bacc.Bacc(None, target_bir_lowering=False, debug=False)

    xT = nc.dram_tensor("xT", [E, T], dt_in, kind="ExternalInput")
    wq = nc.dram_tensor("wq", [E, HD], dt_in, kind="ExternalInput")
    wk = nc.dram_tensor("wk", [E, HD], dt_in, kind="ExternalInput")
    wv = nc.dram_tensor("wv", [E, HD], dt_in, kind="ExternalInput")
    bq = nc.dram_tensor("bq", [HD, 1], F32, kind="ExternalInput")
    bk = nc.dram_tensor("bk", [HD, 1], F32, kind="ExternalInput")
    bv = nc.dram_tensor("bv", [HD, 1], F32, kind="ExternalInput")
    wo = nc.dram_tensor("wo", [HD, E], dt_in, kind="ExternalInput")
    cosT = nc.dram_tensor("cosT", [HD, T], F32, kind="ExternalInput")
    sinT = nc.dram_tensor("sinT", [HD, T], F32, kind="ExternalInput")
    rot = nc.dram_tensor("rot", [HD, HD], dt_in, kind="ExternalInput")
    ones = nc.dram_tensor("ones", [1, 1], dt_in, kind="ExternalInput")
    yp = nc.dram_tensor("yp", [T, E], F32, kind="ExternalOutput")

    scale = 1.0 / np.sqrt(D)

    with tile.TileContext(nc) as tc, ExitStack() as ctx:
        const = ctx.enter_context(tc.tile_pool(name="const", bufs=1))
        xt_pool = ctx.enter_context(tc.tile_pool(name="xt", bufs=KE + 2))
        cs_pool = ctx.enter_context(tc.tile_pool(name="cs", bufs=3))
        qkraw_pool = ctx.enter_context(tc.tile_pool(name="qkraw", bufs=3))
        rope_tmp = ctx.enter_context(tc.tile_pool(name="ropetmp", bufs=3))
        persist = ctx.enter_context(tc.tile_pool(name="persist", bufs=1))
        exps_pool = ctx.enter_context(tc.tile_pool(name="exps", bufs=6))
        zr_pool = ctx.enter_context(tc.tile_pool(name="zr", bufs=4))
        zb_pool = ctx.enter_context(tc.tile_pool(name="zb", bufs=4))
        osb_pool = ctx.enter_context(tc.tile_pool(name="osb", bufs=4))
        csh_pool = ctx.enter_context(tc.tile_pool(name="csh", bufs=3))
        dram = ctx.enter_context(tc.tile_pool(name="dram", bufs=6, space="DRAM"))

        # cross-stage PSUM pools: A = 2 slots x 2 banks, B = 4 slots x 1 bank
        ps_a = ctx.enter_context(tc.tile_pool(name="ps_a", bufs=2, space="PSUM"))
        ps_b = ctx.enter_context(tc.tile_pool(name="ps_b", bufs=4, space="PSUM"))

        # ---- constants to SBUF (gpsimd queue; off the sync DMA path) ----
        def load_const(name, dram_t, shape, dt):
            t = const.tile(shape, dt, name=name, tag=name)
            nc.gpsimd.dma_start(t[:], dram_t.ap())
            return t

        wq_sb = [None] * KE
        wk_sb = [None] * KE
        wv_sb = [None] * KE
        for k in range(KE):
            for nm, dr, arr in (("wq", wq, wq_sb), ("wk", wk, wk_sb), ("wv", wv, wv_sb)):
                t = const.tile([128, HD], dt_in, name=f"{nm}_{k}", tag=f"{nm}_{k}")
                nc.gpsimd.dma_start(t[:], dr.ap()[128 * k : 128 * (k + 1), :])
                arr[k] = t
        bq_sb = load_const("bq_sb", bq, [HD, 1], F32)
        bk_sb = load_const("bk_sb", bk, [HD, 1], F32)
        bv_sb = load_const("bv_sb", bv, [HD, 1], F32)
        wo_sb = load_const("wo_sb", wo, [HD, E], dt_in)
        rot_sb = load_const("rot_sb", rot, [HD, HD], dt_in)
        ident = const.tile([128, 128], dt_in, name="ident", tag="ident")
        make_identity(nc, ident)

        # ---- persistent intermediates ----
        q_rope = persist.tile([HD, T], dt_in, name="q_rope", tag="q_rope")
        k_rope = persist.tile([HD, T], dt_in, name="k_rope", tag="k_rope")
        v_sb = []
        ones_ap = ones.ap()
        for i in range(T // 128):
            t = persist.tile([128, HPC * (D + 1)], dt_in, name=f"v_{i}", tag=f"v_{i}")
            for h in range(HPC):
                nc.gpsimd.dma_start(
                    t[:, (D + 1) * h + D : (D + 1) * (h + 1)],
                    bass.AP(tensor=ones_ap.tensor, offset=ones_ap.offset, ap=[[0, 128], [1, 1]]),
                )
            v_sb.append(t)
        ctx_pack = {}
        for b in range(B):
            ctx_pack[b] = persist.tile([HD, S], dt_in, name=f"ctxp_{b}", tag=f"ctxp_{b}")

        # ---- stage 1: projections (qT/kT/vT) + RoPE + v transpose ----
        for c in range(NCH):
            c0 = TC * c
            xt = [None] * KE
            for k in range(KE):
                t = xt_pool.tile([128, TC], dt_in, name=f"xt_{c}_{k}", tag="xt")
                nc.sync.dma_start(t[:], xT.ap()[128 * k : 128 * (k + 1), c0 : c0 + TC])
                xt[k] = t
            cos_c = cs_pool.tile([HD, TC], F32, name="cos_c", tag="cos_c")
            nc.scalar.dma_start(cos_c[:], cosT.ap()[:, c0 : c0 + TC])
            sin_c = cs_pool.tile([HD, TC], F32, name="sin_c", tag="sin_c")
            nc.scalar.dma_start(sin_c[:], sinT.ap()[:, c0 : c0 + TC])

            psq = ps_a.tile([HD, TC], F32, name="psq", tag="ps_a")
            psk = ps_a.tile([HD, TC], F32, name="psk", tag="ps_a")
            psv = ps_b.tile([HD, TC], F32, name="psv", tag="ps_b")
            for ps, w_sb in ((psq, wq_sb), (psk, wk_sb), (psv, wv_sb)):
                for k in range(KE):
                    nc.tensor.matmul(
                        ps[:], w_sb[k][:], xt[k][:],
                        start=(k == 0), stop=(k == KE - 1),
                    )
            # vT -> v natural via PE transpose; bias evict on ACT
            vraw = qkraw_pool.tile([HD, TC], dt_in, name="vraw", tag="qkraw")
            nc.scalar.activation(vraw[:], psv[:], AF.Identity, bias=bv_sb[:, 0:1])
            for j in range(TC // 128):
                pvt = ps_b.tile([128, 128], dt_in, name="pvt", tag="ps_b")
                nc.tensor.transpose(pvt[:], vraw[:, 128 * j : 128 * (j + 1)], ident[:])
                vt = v_sb[(c0 + 128 * j) // 128]
                for h in range(HPC):
                    nc.vector.tensor_copy(
                        vt[:, (D + 1) * h : (D + 1) * h + D],
                        pvt[:, D * h : D * (h + 1)],
                    )
            # q/k: bias evict on ACT, rotate matmul, rope combine DVE+GpSimd
            for nm, ps, b_sb, out in (
                ("q", psq, bq_sb, q_rope),
                ("k", psk, bk_sb, k_rope),
            ):
                raw = qkraw_pool.tile([HD, TC], dt_in, name=f"{nm}raw", tag="qkraw")
                nc.scalar.activation(raw[:], ps[:], AF.Identity, bias=b_sb[:, 0:1])
                psrot = ps_b.tile([HD, TC], F32, name="psrot", tag="ps_b")
                nc.tensor.matmul(psrot[:], rot_sb[:], raw[:], start=True, stop=True)
                sprod = rope_tmp.tile([HD, TC], F32, name="sprod", tag="ropetmp")
                nc.vector.tensor_tensor(sprod[:], psrot[:], sin_c[:], op=OP.mult)
                cprod = rope_tmp.tile([HD, TC], F32, name="cprod", tag="ropetmp")
                nc.vector.tensor_tensor(cprod[:], raw[:], cos_c[:], op=OP.mult)
                nc.gpsimd.tensor_tensor(
                    out[:, c0 : c0 + TC], cprod[:], sprod[:], op=OP.add
                )

        # ---- stage 2: attention ----
        for b in range(B):
            t0 = b * S
            for qc in range(NQC):
                q0 = t0 + QC * qc
                psc = [
                    ps_b.tile([D + 1, QC], F32, name=f"psctx{h}", tag="ps_b")
                    for h in range(HPC)
                ]
                for kt in range(NKT):
                    k0 = t0 + 128 * kt
                    # both heads' scores into one 2-bank psum tile -> single exp
                    pss = ps_a.tile([128, HPC * QC], F32, name="pss", tag="ps_a")
                    for h in range(HPC):
                        nc.tensor.matmul(
                            pss[:, QC * h : QC * (h + 1)],
                            k_rope[D * h : D * (h + 1), k0 : k0 + 128],
                            q_rope[D * h : D * (h + 1), q0 : q0 + QC],
                            start=True, stop=True,
                        )
                    ex = exps_pool.tile([128, HPC * QC], dt_in, name="ex", tag="exps")
                    nc.scalar.activation(ex[:], pss[:], AF.Exp, scale=scale)
                    for h in range(HPC):
                        nc.tensor.matmul(
                            psc[h][:],
                            v_sb[k0 // 128][:, (D + 1) * h : (D + 1) * (h + 1)],
                            ex[:, QC * h : QC * (h + 1)],
                            start=(kt == 0), stop=(kt == NKT - 1),
                        )
                # normalize: 1/Z on DVE, broadcast via DRAM bounce, multiply
                for h in range(HPC):
                    zr = zr_pool.tile([1, QC], F32, name="zrec", tag="zr")
                    nc.vector.reciprocal(zr[:], psc[h][D : D + 1, :])
                    zd = dram.tile([QC], F32, name="zd", tag="zd")
                    nc.gpsimd.dma_start(zd[:], zr[:])
                    zb = zb_pool.tile([D, QC], F32, name="zb", tag="zb")
                    zd_ap = zd[:]
                    nc.gpsimd.dma_start(
                        zb[:],
                        bass.AP(
                            tensor=zd_ap.tensor, offset=zd_ap.offset,
                            ap=[[0, D], [1, QC]],
                        ),
                    )
                    if h == 0:
                        nc.vector.tensor_tensor(
                            ctx_pack[b][0:D, QC * qc : QC * (qc + 1)],
                            psc[h][0:D, :], zb[:], op=OP.mult,
                        )
                    else:
                        csh = csh_pool.tile([D, QC], dt_in, name="csh", tag="csh")
                        nc.vector.tensor_tensor(csh[:], psc[h][0:D, :], zb[:], op=OP.mult)
                        nc.scalar.dma_start(
                            ctx_pack[b][D : 2 * D, QC * qc : QC * (qc + 1)], csh[:]
                        )

        # ---- stage 3: output projection (partial: this core's Wo rows) ----
        for b in range(B):
            t0 = b * S
            for j in range(S // 128):
                for e in range(E // 512):
                    pso = ps_b.tile([128, 512], F32, name="pso", tag="ps_b")
                    nc.tensor.matmul(
                        pso[:],
                        ctx_pack[b][:, 128 * j : 128 * (j + 1)],
                        wo_sb[:, 512 * e : 512 * (e + 1)],
                        start=True, stop=True,
                    )
                    osb = osb_pool.tile([128, 512], F32, name="osb", tag="osb")
                    if e == 0:
                        nc.vector.tensor_copy(osb[:], pso[:])
                    else:
                        nc.scalar.activation(osb[:], pso[:], AF.Copy)
                    eng = nc.sync if e == 0 else nc.gpsimd
                    eng.dma_start(
                        yp.ap()[t0 + 128 * j : t0 + 128 * (j + 1), 512 * e : 512 * (e + 1)],
                        osb[:],
                    )

    nc.compile()
    return nc


def _rope_tables():
    inv_freq = 1.0 / (10000.0 ** (np.arange(0, D, 2, dtype=np.float32) / D))
    t = np.arange(S, dtype=np.float32)
    freqs = np.outer(t, inv_freq).astype(np.float32)
    emb = np.concatenate([freqs, freqs], axis=-1)
    return np.cos(emb).astype(np.float32), np.sin(emb).astype(np.float32)


def _rot_matrix():
    R = np.zeros((HD, HD), np.float32)
    for hh in range(HPC):
        for do in range(D):
            po = D * hh + do
            if do < D // 2:
                R[D * hh + do + D // 2, po] = -1.0
            else:
                R[D * hh + do - D // 2, po] = 1.0
    return R


def kernel(x, Wq, bq, Wk, bk, Wv, bv, Wo, bo):
    global LAST_RESULTS
    import ml_dtypes

    x = np.asarray(x, dtype=np.float32)
    Wq, bq = np.asarray(Wq, np.float32), np.asarray(bq, np.float32)
    Wk, bk = np.asarray(Wk, np.float32), np.asarray(bk, np.float32)
    Wv, bv = np.asarray(Wv, np.float32), np.asarray(bv, np.float32)
    Wo, bo = np.asarray(Wo, np.float32), np.asarray(bo, np.float32)

    mode = MM_MODE
    dt_np = ml_dtypes.bfloat16 if mode == "bf16" else np.float32
    T = B * S

    if mode not in _NC_CACHE:
        _NC_CACHE[mode] = build_mha_nc(mode)
    nc = _NC_CACHE[mode]

    xT = np.ascontiguousarray(x.reshape(T, E).T).astype(dt_np)
    cos, sin = _rope_tables()
    cosT = np.tile(np.ascontiguousarray(cos.T), (HPC, B)).astype(np.float32)
    sinT = np.tile(np.ascontiguousarray(sin.T), (HPC, B)).astype(np.float32)
    R = _rot_matrix().astype(dt_np)

    in_maps = []
    for c in range(N_CORES):
        sl = slice(HD * c, HD * (c + 1))
        in_maps.append(
            {
                "xT": xT,
                "wq": np.ascontiguousarray(Wq[:, sl]).astype(dt_np),
                "wk": np.ascontiguousarray(Wk[:, sl]).astype(dt_np),
                "wv": np.ascontiguousarray(Wv[:, sl]).astype(dt_np),
                "bq": np.ascontiguousarray(bq[sl][:, None]).astype(np.float32),
                "bk": np.ascontiguousarray(bk[sl][:, None]).astype(np.float32),
                "bv": np.ascontiguousarray(bv[sl][:, None]).astype(np.float32),
                "wo": np.ascontiguousarray(Wo[sl, :]).astype(dt_np),
                "cosT": cosT,
                "sinT": sinT,
                "rot": R,
                "ones": np.ones((1, 1), dt_np),
            }
        )

    res = bass_utils.run_bass_kernel_spmd(nc, in_maps, core_ids=list(range(N_CORES)))
    LAST_RESULTS = res

    out = np.zeros((T, E), np.float64)
    for c in range(N_CORES):
        out += res.results[c]["yp"].astype(np.float64)
    out += bo.astype(np.float64)
    return out.astype(np.float32).reshape(B, S, E)


# revision 10
# speedup vs baseline: 3.0820x; 1.2828x over previous
"""Trainium2 Bass kernel for nn_MultiHeadAttention (RoPE MHA, B=2 S=2048 E=1024 H=16).

Sharding: tensor-parallel over heads — 2 heads per core on 8 cores. Each core
computes its heads' q/k/v projections, RoPE, attention, and the partial output
projection (its rows of Wo); the host sums the 8 partials and adds bo.

Device layouts: q/k as [d, token] (transposed) so attention scores come out as
[ks, qs]; softmax's row-sum falls out of the same matmul that computes ctx via
a ones column appended to v. Normalization multiplies by 1/Z broadcast across
partitions via a DRAM bounce. rotate_half is a signed-permutation matmul.
v is projected transposed (full-width matmuls) and PE-transposed to natural.

Engine balance: exp on ACT (the stage-2 pacer), projection evictions on ACT,
RoPE products + normalize on DVE, RoPE final add on GpSimd, DMAs spread over
sync/scalar/vector/gpsimd queues. Two cross-stage PSUM pools (no stage
barriers): A = 2x2-bank slots (psq/psk/pss), B = 4x1-bank slots (rest).
"""

import os
import sys
from contextlib import ExitStack

import numpy as np

for _p in ("/opt/trn_rl_repo", "/opt/pypackages"):
    if _p not in sys.path and os.path.isdir(_p):
        sys.path.append(_p)

import concourse.bass as bass
import concourse.mybir as mybir
import concourse.tile as tile
from concourse import bacc
from concourse import bass_utils
from concourse.masks import make_identity

F32 = mybir.dt.float32
AF = mybir.ActivationFunctionType
OP = mybir.AluOpType

B = 2
S = 2048
E = 1024
H = 16
D = 64
N_CORES = 8
HPC = H // N_CORES  # heads per core = 2
HD = HPC * D  # 128

MM_MODE = os.environ.get("MHA_MM_MODE", "bf16")  # 'f32' | 'f32r' | 'bf16'

LAST_RESULTS = None  # BassKernelResults of the most recent run (for test harness)
_NC_CACHE = {}


def build_mha_nc(mm_mode):
    T = B * S
    TC = 512  # token chunk for projections
    NCH = T // TC
    QC = min(512, S)  # query chunk in attention (N<=512: one PSUM bank per matmul)
    NQC = S // QC
    NKT = S // 128  # key tiles per batch
    KE = E // 128  # contraction tiles for projections

    dt_in = {"bf16": mybir.dt.bfloat16, "f32r": mybir.dt.float32r, "f32": F32}[mm_mode]

    nc = bacc.Bacc(None, target_bir_lowering=False, debug=False)

    xT = nc.dram_tensor("xT", [E, T], dt_in, kind="ExternalInput")
    wq = nc.dram_tensor("wq", [E, HD], dt_in, kind="ExternalInput")
    wk = nc.dram_tensor("wk", [E, HD], dt_in, kind="ExternalInput")
    wv = nc.dram_tensor("wv", [E, HD], dt_in, kind="ExternalInput")
    bq = nc.dram_tensor("bq", [HD, 1], F32, kind="ExternalInput")
    bk = nc.dram_tensor("bk", [HD, 1], F32, kind="ExternalInput")
    bv = nc.dram_tensor("bv", [HD, 1], F32, kind="ExternalInput")
    wo = nc.dram_tensor("wo", [HD, E], dt_in, kind="ExternalInput")
    cosT = nc.dram_tensor("cosT", [HD, T], F32, kind="ExternalInput")
    sinT = nc.dram_tensor("sinT", [HD, T], F32, kind="ExternalInput")
    rot = nc.dram_tensor("rot", [HD, HD], dt_in, kind="ExternalInput")
    ones = nc.dram_tensor("ones", [1, 1], dt_in, kind="ExternalInput")
    yp = nc.dram_tensor("yp", [T, E], F32, kind="ExternalOutput")

    scale = 1.0 / np.sqrt(D)

    with tile.TileContext(nc) as tc, ExitStack() as ctx:
        const = ctx.enter_context(tc.tile_pool(name="const", bufs=1))
        xt_pool = ctx.enter_context(tc.tile_pool(name="xt", bufs=KE + 2))
        cs_pool = ctx.enter_context(tc.tile_pool(name="cs", bufs=3))
        qkraw_pool = ctx.enter_context(tc.tile_pool(name="qkraw", bufs=3))
        rope_tmp = ctx.enter_context(tc.tile_pool(name="ropetmp", bufs=3))
        persist = ctx.enter_context(tc.tile_pool(name="persist", bufs=1))
        exps_pool = ctx.enter_context(tc.tile_pool(name="exps", bufs=6))
        zr_pool = ctx.enter_context(tc.tile_pool(name="zr", bufs=4))
        zb_pool = ctx.enter_context(tc.tile_pool(name="zb", bufs=4))
        osb_pool = ctx.enter_context(tc.tile_pool(name="osb", bufs=4))
        csh_pool = ctx.enter_context(tc.tile_pool(name="csh", bufs=3))
        dram = ctx.enter_context(tc.tile_pool(name="dram", bufs=6, space="DRAM"))

        # cross-stage PSUM pools: A = 2 slots x 2 banks, B = 4 slots x 1 bank
        ps_a = ctx.enter_context(tc.tile_pool(name="ps_a", bufs=2, space="PSUM"))
        ps_b = ctx.enter_context(tc.tile_pool(name="ps_b", bufs=4, space="PSUM"))

        # ---- constants to SBUF (gpsimd queue; off the sync DMA path) ----
        def load_const(name, dram_t, shape, dt):
            t = const.tile(shape, dt, name=name, tag=name)
            nc.gpsimd.dma_start(t[:], dram_t.ap())
            return t

        wq_sb = [None] * KE
        wk_sb = [None] * KE
        wv_sb = [None] * KE
        for k in range(KE):
            for nm, dr, arr in (("wq", wq, wq_sb), ("wk", wk, wk_sb), ("wv", wv, wv_sb)):
                t = const.tile([128, HD], dt_in, name=f"{nm}_{k}", tag=f"{nm}_{k}")
                nc.gpsimd.dma_start(t[:], dr.ap()[128 * k : 128 * (k + 1), :])
                arr[k] = t
        bq_sb = load_const("bq_sb", bq, [HD, 1], F32)
        bk_sb = load_const("bk_sb", bk, [HD, 1], F32)
        bv_sb = load_const("bv_sb", bv, [HD, 1], F32)
        wo_sb = load_const("wo_sb", wo, [HD, E], dt_in)
        rot_sb = load_const("rot_sb", rot, [HD, HD], dt_in)
        ident = const.tile([128, 128], dt_in, name="ident", tag="ident")
        make_identity(nc, ident)

        # ---- persistent intermediates ----
        q_rope = persist.tile([HD, T], dt_in, name="q_rope", tag="q_rope")
        k_rope = persist.tile([HD, T], dt_in, name="k_rope", tag="k_rope")
        v_sb = []
        ones_ap = ones.ap()
        for i in range(T // 128):
            t = persist.tile([128, HPC * (D + 1)], dt_in, name=f"v_{i}", tag=f"v_{i}")
            for h in range(HPC):
                sl_ones = t[:, (D + 1) * h + D : (D + 1) * (h + 1)]
                if mm_mode == "f32r":
                    nc.gpsimd.dma_start(
                        sl_ones,
                        bass.AP(tensor=ones_ap.tensor, offset=ones_ap.offset, ap=[[0, 128], [1, 1]]),
                    )
                else:
                    nc.vector.memset(sl_ones, 1.0)
            v_sb.append(t)
        ctx_pack = {}
        for b in range(B):
            ctx_pack[b] = persist.tile([HD, S], dt_in, name=f"ctxp_{b}", tag=f"ctxp_{b}")

        # ---- stage 1: projections (qT/kT/vT) + RoPE + v transpose ----
        for c in range(NCH):
            c0 = TC * c
            xt = [None] * KE
            for k in range(KE):
                t = xt_pool.tile([128, TC], dt_in, name=f"xt_{c}_{k}", tag="xt")
                nc.sync.dma_start(t[:], xT.ap()[128 * k : 128 * (k + 1), c0 : c0 + TC])
                xt[k] = t
            cos_c = cs_pool.tile([HD, TC], F32, name="cos_c", tag="cos_c")
            nc.scalar.dma_start(cos_c[:], cosT.ap()[:, c0 : c0 + TC])
            sin_c = cs_pool.tile([HD, TC], F32, name="sin_c", tag="sin_c")
            nc.scalar.dma_start(sin_c[:], sinT.ap()[:, c0 : c0 + TC])

            psq = ps_a.tile([HD, TC], F32, name="psq", tag="ps_a")
            psk = ps_a.tile([HD, TC], F32, name="psk", tag="ps_a")
            psv = ps_b.tile([HD, TC], F32, name="psv", tag="ps_b")
            for ps, w_sb in ((psq, wq_sb), (psk, wk_sb), (psv, wv_sb)):
                for k in range(KE):
                    nc.tensor.matmul(
                        ps[:], w_sb[k][:], xt[k][:],
                        start=(k == 0), stop=(k == KE - 1),
                    )
            # vT -> v natural via PE transpose; bias evict on ACT
            vraw = qkraw_pool.tile([HD, TC], dt_in, name="vraw", tag="qkraw")
            nc.scalar.activation(vraw[:], psv[:], AF.Identity, bias=bv_sb[:, 0:1])
            for j in range(TC // 128):
                pvt = ps_b.tile([128, 128], dt_in, name="pvt", tag="ps_b")
                nc.tensor.transpose(pvt[:], vraw[:, 128 * j : 128 * (j + 1)], ident[:])
                vt = v_sb[(c0 + 128 * j) // 128]
                for h in range(HPC):
                    nc.vector.tensor_copy(
                        vt[:, (D + 1) * h : (D + 1) * h + D],
                        pvt[:, D * h : D * (h + 1)],
                    )
            # q/k: bias evict on ACT, rotate matmul, rope combine DVE+GpSimd
            for nm, ps, b_sb, out in (
                ("q", psq, bq_sb, q_rope),
                ("k", psk, bk_sb, k_rope),
            ):
                raw = qkraw_pool.tile([HD, TC], dt_in, name=f"{nm}raw", tag="qkraw")
                nc.scalar.activation(raw[:], ps[:], AF.Identity, bias=b_sb[:, 0:1])
                psrot = ps_b.tile([HD, TC], F32, name="psrot", tag="ps_b")
                nc.tensor.matmul(psrot[:], rot_sb[:], raw[:], start=True, stop=True)
                sprod = rope_tmp.tile([HD, TC], F32, name="sprod", tag="ropetmp")
                nc.vector.tensor_tensor(sprod[:], psrot[:], sin_c[:], op=OP.mult)
                cprod = rope_tmp.tile([HD, TC], F32, name="cprod", tag="ropetmp")
                nc.vector.tensor_tensor(cprod[:], raw[:], cos_c[:], op=OP.mult)
                nc.gpsimd.tensor_tensor(
                    out[:, c0 : c0 + TC], cprod[:], sprod[:], op=OP.add
                )

        # ---- stage 3 emitter (called interleaved with stage 2) ----
        def emit_out_proj(b, tiles):
            t0 = b * S
            for j in tiles:
                for e in range(E // 512):
                    pso = ps_b.tile([128, 512], F32, name="pso", tag="ps_b")
                    nc.tensor.matmul(
                        pso[:],
                        ctx_pack[b][:, 128 * j : 128 * (j + 1)],
                        wo_sb[:, 512 * e : 512 * (e + 1)],
                        start=True, stop=True,
                    )
                    osb = osb_pool.tile([128, 512], F32, name="osb", tag="osb")
                    nc.vector.tensor_copy(osb[:], pso[:])
                    eng = nc.sync if e == 0 else nc.gpsimd
                    eng.dma_start(
                        yp.ap()[t0 + 128 * j : t0 + 128 * (j + 1), 512 * e : 512 * (e + 1)],
                        osb[:],
                    )

        # ---- stage 2: attention (stage-3 of batch b-1 interleaved) ----
        JT = S // 128  # out-proj tiles per batch
        for b in range(B):
            t0 = b * S
            for qc in range(NQC):
                if b > 0:
                    emit_out_proj(b - 1, range(JT * qc // NQC, JT * (qc + 1) // NQC))
                q0 = t0 + QC * qc
                psc = [
                    ps_b.tile([D + 1, QC], F32, name=f"psctx{h}", tag="ps_b")
                    for h in range(HPC)
                ]
                for kt in range(NKT):
                    k0 = t0 + 128 * kt
                    # both heads' scores into one 2-bank psum tile -> single exp
                    pss = ps_a.tile([128, HPC * QC], F32, name="pss", tag="ps_a")
                    for h in range(HPC):
                        nc.tensor.matmul(
                            pss[:, QC * h : QC * (h + 1)],
                            k_rope[D * h : D * (h + 1), k0 : k0 + 128],
                            q_rope[D * h : D * (h + 1), q0 : q0 + QC],
                            start=True, stop=True,
                        )
                    ex = exps_pool.tile([128, HPC * QC], dt_in, name="ex", tag="exps")
                    nc.scalar.activation(ex[:], pss[:], AF.Exp, scale=scale)
                    for h in range(HPC):
                        nc.tensor.matmul(
                            psc[h][:],
                            v_sb[k0 // 128][:, (D + 1) * h : (D + 1) * (h + 1)],
                            ex[:, QC * h : QC * (h + 1)],
                            start=(kt == 0), stop=(kt == NKT - 1),
                        )
                # normalize: 1/Z on DVE, broadcast via DRAM bounce, multiply
                for h in range(HPC):
                    zr = zr_pool.tile([1, QC], F32, name="zrec", tag="zr")
                    nc.vector.reciprocal(zr[:], psc[h][D : D + 1, :])
                    zd = dram.tile([QC], F32, name="zd", tag="zd")
                    nc.gpsimd.dma_start(zd[:], zr[:])
                    zb = zb_pool.tile([D, QC], F32, name="zb", tag="zb")
                    zd_ap = zd[:]
                    nc.gpsimd.dma_start(
                        zb[:],
                        bass.AP(
                            tensor=zd_ap.tensor, offset=zd_ap.offset,
                            ap=[[0, D], [1, QC]],
                        ),
                    )
                    if h == 0:
                        nc.vector.tensor_tensor(
                            ctx_pack[b][0:D, QC * qc : QC * (qc + 1)],
                            psc[h][0:D, :], zb[:], op=OP.mult,
                        )
                    else:
                        csh = csh_pool.tile([D, QC], dt_in, name="csh", tag="csh")
                        nc.vector.tensor_tensor(csh[:], psc[h][0:D, :], zb[:], op=OP.mult)
                        nc.scalar.dma_start(
                            ctx_pack[b][D : 2 * D, QC * qc : QC * (qc + 1)], csh[:]
                        )

        # ---- stage 3: remaining output projection (last batch) ----
        emit_out_proj(B - 1, range(JT))

    nc.compile()
    return nc


def _rope_tables():
    inv_freq = 1.0 / (10000.0 ** (np.arange(0, D, 2, dtype=np.float32) / D))
    t = np.arange(S, dtype=np.float32)
    freqs = np.outer(t, inv_freq).astype(np.float32)
    emb = np.concatenate([freqs, freqs], axis=-1)
    return np.cos(emb).astype(np.float32), np.sin(emb).astype(np.float32)


def _rot_matrix():
    R = np.zeros((HD, HD), np.float32)
    for hh in range(HPC):
        for do in range(D):
            po = D * hh + do
            if do < D // 2:
                R[D * hh + do + D // 2, po] = -1.0
            else:
                R[D * hh + do - D // 2, po] = 1.0
    return R


def kernel(x, Wq, bq, Wk, bk, Wv, bv, Wo, bo):
    global LAST_RESULTS
    import ml_dtypes

    x = np.asarray(x, dtype=np.float32)
    Wq, bq = np.asarray(Wq, np.float32), np.asarray(bq, np.float32)
    Wk, bk = np.asarray(Wk, np.float32), np.asarray(bk, np.float32)
    Wv, bv = np.asarray(Wv, np.float32), np.asarray(bv, np.float32)
    Wo, bo = np.asarray(Wo, np.float32), np.asarray(bo, np.float32)

    mode = MM_MODE
    dt_np = ml_dtypes.bfloat16 if mode == "bf16" else np.float32
    T = B * S

    if mode not in _NC_CACHE:
        _NC_CACHE[mode] = build_mha_nc(mode)
    nc = _NC_CACHE[mode]

    xT = np.ascontiguousarray(x.reshape(T, E).T).astype(dt_np)
    cos, sin = _rope_tables()
    cosT = np.tile(np.ascontiguousarray(cos.T), (HPC, B)).astype(np.float32)
    sinT = np.tile(np.ascontiguousarray(sin.T), (HPC, B)).astype(np.float32)
    R = _rot_matrix().astype(dt_np)

    in_maps = []
    for c in range(N_CORES):
        sl = slice(HD * c, HD * (c + 1))
        in_maps.append(
            {
                "xT": xT,
                "wq": np.ascontiguousarray(Wq[:, sl]).astype(dt_np),
                "wk": np.ascontiguousarray(Wk[:, sl]).astype(dt_np),
                "wv": np.ascontiguousarray(Wv[:, sl]).astype(dt_np),
                "bq": np.ascontiguousarray(bq[sl][:, None]).astype(np.float32),
                "bk": np.ascontiguousarray(bk[sl][:, None]).astype(np.float32),
                "bv": np.ascontiguousarray(bv[sl][:, None]).astype(np.float32),
                "wo": np.ascontiguousarray(Wo[sl, :]).astype(dt_np),
                "cosT": cosT,
                "sinT": sinT,
                "rot": R,
                "ones": np.ones((1, 1), dt_np),
            }
        )

    res = bass_utils.run_bass_kernel_spmd(nc, in_maps, core_ids=list(range(N_CORES)))
    LAST_RESULTS = res

    out = np.zeros((T, E), np.float64)
    for c in range(N_CORES):
        out += res.results[c]["yp"].astype(np.float64)
    out += bo.astype(np.float64)
    return out.astype(np.float32).reshape(B, S, E)


# revision 12
# speedup vs baseline: 3.1184x; 1.0118x over previous
"""Trainium2 Bass kernel for nn_MultiHeadAttention (RoPE MHA, B=2 S=2048 E=1024 H=16).

Sharding: tensor-parallel over heads — 2 heads per core on 8 cores. Each core
computes its heads' q/k/v projections, RoPE, attention, and the partial output
projection (its rows of Wo); the host sums the 8 partials and adds bo.

Device layouts: q/k as [d, token] (transposed) so attention scores come out as
[ks, qs]; softmax's row-sum falls out of the same matmul that computes ctx via
a ones column appended to v. Normalization multiplies by 1/Z broadcast across
partitions via a DRAM bounce. rotate_half is a signed-permutation matmul.
v is projected transposed (full-width matmuls) and PE-transposed to natural.

Engine balance: exp on ACT (the stage-2 pacer), projection evictions on ACT,
RoPE products + normalize on DVE, RoPE final add on GpSimd, DMAs spread over
sync/scalar/vector/gpsimd queues. Two cross-stage PSUM pools (no stage
barriers): A = 2x2-bank slots (psq/psk/pss), B = 4x1-bank slots (rest).
"""

import os
import sys
from contextlib import ExitStack

import numpy as np

for _p in ("/opt/trn_rl_repo", "/opt/pypackages"):
    if _p not in sys.path and os.path.isdir(_p):
        sys.path.append(_p)

import concourse.bass as bass
import concourse.mybir as mybir
import concourse.tile as tile
from concourse import bacc
from concourse import bass_utils
from concourse.masks import make_identity

F32 = mybir.dt.float32
AF = mybir.ActivationFunctionType
OP = mybir.AluOpType

B = 2
S = 2048
E = 1024
H = 16
D = 64
N_CORES = 8
HPC = H // N_CORES  # heads per core = 2
HD = HPC * D  # 128

MM_MODE = os.environ.get("MHA_MM_MODE", "bf16")  # 'f32' | 'f32r' | 'bf16'

LAST_RESULTS = None  # BassKernelResults of the most recent run (for test harness)
_NC_CACHE = {}


def build_mha_nc(mm_mode):
    T = B * S
    TC = 512  # token chunk for projections
    NCH = T // TC
    QC = min(512, S)  # query chunk in attention (N<=512: one PSUM bank per matmul)
    NQC = S // QC
    NKT = S // 128  # key tiles per batch
    KE = E // 128  # contraction tiles for projections

    dt_in = {"bf16": mybir.dt.bfloat16, "f32r": mybir.dt.float32r, "f32": F32}[mm_mode]

    nc = bacc.Bacc(None, target_bir_lowering=False, debug=False)

    xT = nc.dram_tensor("xT", [E, T], dt_in, kind="ExternalInput")
    wq = nc.dram_tensor("wq", [E, HD], dt_in, kind="ExternalInput")
    wk = nc.dram_tensor("wk", [E, HD], dt_in, kind="ExternalInput")
    wv = nc.dram_tensor("wv", [E, HD], dt_in, kind="ExternalInput")
    bq = nc.dram_tensor("bq", [HD, 1], F32, kind="ExternalInput")
    bk = nc.dram_tensor("bk", [HD, 1], F32, kind="ExternalInput")
    bv = nc.dram_tensor("bv", [HD, 1], F32, kind="ExternalInput")
    wo = nc.dram_tensor("wo", [HD, E], dt_in, kind="ExternalInput")
    cosT = nc.dram_tensor("cosT", [HD, T], F32, kind="ExternalInput")
    sinT = nc.dram_tensor("sinT", [HD, T], F32, kind="ExternalInput")
    rot = nc.dram_tensor("rot", [HD, HD], dt_in, kind="ExternalInput")
    ones = nc.dram_tensor("ones", [1, 1], dt_in, kind="ExternalInput")
    yp = nc.dram_tensor("yp", [T, E], F32, kind="ExternalOutput")

    scale = 1.0 / np.sqrt(D)

    with tile.TileContext(nc) as tc, ExitStack() as ctx:
        const = ctx.enter_context(tc.tile_pool(name="const", bufs=1))
        xt_pool = ctx.enter_context(tc.tile_pool(name="xt", bufs=KE + 2))
        cs_pool = ctx.enter_context(tc.tile_pool(name="cs", bufs=3))
        qkraw_pool = ctx.enter_context(tc.tile_pool(name="qkraw", bufs=3))
        rope_tmp = ctx.enter_context(tc.tile_pool(name="ropetmp", bufs=3))
        persist = ctx.enter_context(tc.tile_pool(name="persist", bufs=1))
        exps_pool = ctx.enter_context(tc.tile_pool(name="exps", bufs=6))
        zr_pool = ctx.enter_context(tc.tile_pool(name="zr", bufs=4))
        zb_pool = ctx.enter_context(tc.tile_pool(name="zb", bufs=4))
        osb_pool = ctx.enter_context(tc.tile_pool(name="osb", bufs=4))
        csh_pool = ctx.enter_context(tc.tile_pool(name="csh", bufs=3))
        dram = ctx.enter_context(tc.tile_pool(name="dram", bufs=6, space="DRAM"))

        # cross-stage PSUM pools: A = 2 slots x 2 banks, B = 4 slots x 1 bank
        ps_a = ctx.enter_context(tc.tile_pool(name="ps_a", bufs=2, space="PSUM"))
        ps_b = ctx.enter_context(tc.tile_pool(name="ps_b", bufs=4, space="PSUM"))

        # ---- constants to SBUF (gpsimd queue; off the sync DMA path) ----
        def load_const(name, dram_t, shape, dt):
            t = const.tile(shape, dt, name=name, tag=name)
            nc.gpsimd.dma_start(t[:], dram_t.ap())
            return t

        wq_sb = [None] * KE
        wk_sb = [None] * KE
        wv_sb = [None] * KE
        for k in range(KE):
            for nm, dr, arr in (("wq", wq, wq_sb), ("wk", wk, wk_sb), ("wv", wv, wv_sb)):
                t = const.tile([128, HD], dt_in, name=f"{nm}_{k}", tag=f"{nm}_{k}")
                nc.gpsimd.dma_start(t[:], dr.ap()[128 * k : 128 * (k + 1), :])
                arr[k] = t
        bq_sb = load_const("bq_sb", bq, [HD, 1], F32)
        bk_sb = load_const("bk_sb", bk, [HD, 1], F32)
        bv_sb = load_const("bv_sb", bv, [HD, 1], F32)
        wo_sb = load_const("wo_sb", wo, [HD, E], dt_in)
        rot_sb = load_const("rot_sb", rot, [HD, HD], dt_in)
        ident = const.tile([128, 128], dt_in, name="ident", tag="ident")
        make_identity(nc, ident)

        # ---- persistent intermediates ----
        q_rope = persist.tile([HD, T], dt_in, name="q_rope", tag="q_rope")
        k_rope = persist.tile([HD, T], dt_in, name="k_rope", tag="k_rope")
        v_sb = []
        ones_ap = ones.ap()
        for i in range(T // 128):
            t = persist.tile([128, HPC * (D + 1)], dt_in, name=f"v_{i}", tag=f"v_{i}")
            for h in range(HPC):
                sl_ones = t[:, (D + 1) * h + D : (D + 1) * (h + 1)]
                if mm_mode == "f32r":
                    nc.gpsimd.dma_start(
                        sl_ones,
                        bass.AP(tensor=ones_ap.tensor, offset=ones_ap.offset, ap=[[0, 128], [1, 1]]),
                    )
                else:
                    nc.vector.memset(sl_ones, 1.0)
            v_sb.append(t)
        ctx_pack = {}
        for b in range(B):
            ctx_pack[b] = persist.tile([HD, S], dt_in, name=f"ctxp_{b}", tag=f"ctxp_{b}")

        # ---- stage 1 chunk: projections (qT/kT/vT) + RoPE + v transpose ----
        def proj_chunk(c):
            c0 = TC * c
            xt = [None] * KE
            for k in range(KE):
                t = xt_pool.tile([128, TC], dt_in, name=f"xt_{c}_{k}", tag="xt")
                eng = nc.sync if k % 2 == 0 else nc.scalar
                eng.dma_start(t[:], xT.ap()[128 * k : 128 * (k + 1), c0 : c0 + TC])
                xt[k] = t
            cos_c = cs_pool.tile([HD, TC], F32, name="cos_c", tag="cos_c")
            nc.scalar.dma_start(cos_c[:], cosT.ap()[:, c0 : c0 + TC])
            sin_c = cs_pool.tile([HD, TC], F32, name="sin_c", tag="sin_c")
            nc.scalar.dma_start(sin_c[:], sinT.ap()[:, c0 : c0 + TC])

            psq = ps_a.tile([HD, TC], F32, name="psq", tag="ps_a")
            psk = ps_a.tile([HD, TC], F32, name="psk", tag="ps_a")
            psv = ps_b.tile([HD, TC], F32, name="psv", tag="ps_b")
            for ps, w_sb in ((psq, wq_sb), (psk, wk_sb), (psv, wv_sb)):
                for k in range(KE):
                    nc.tensor.matmul(
                        ps[:], w_sb[k][:], xt[k][:],
                        start=(k == 0), stop=(k == KE - 1),
                    )
            # vT -> v natural via PE transpose; bias evict on ACT
            vraw = qkraw_pool.tile([HD, TC], dt_in, name="vraw", tag="qkraw")
            nc.scalar.activation(vraw[:], psv[:], AF.Identity, bias=bv_sb[:, 0:1])
            for j in range(TC // 128):
                pvt = ps_b.tile([128, 128], dt_in, name="pvt", tag="ps_b")
                nc.tensor.transpose(pvt[:], vraw[:, 128 * j : 128 * (j + 1)], ident[:])
                vt = v_sb[(c0 + 128 * j) // 128]
                for h in range(HPC):
                    nc.vector.tensor_copy(
                        vt[:, (D + 1) * h : (D + 1) * h + D],
                        pvt[:, D * h : D * (h + 1)],
                    )
            # q/k: bias evict on ACT, rotate matmul, rope combine DVE+GpSimd
            for nm, ps, b_sb, out in (
                ("q", psq, bq_sb, q_rope),
                ("k", psk, bk_sb, k_rope),
            ):
                raw = qkraw_pool.tile([HD, TC], dt_in, name=f"{nm}raw", tag="qkraw")
                nc.scalar.activation(raw[:], ps[:], AF.Identity, bias=b_sb[:, 0:1])
                psrot = ps_b.tile([HD, TC], F32, name="psrot", tag="ps_b")
                nc.tensor.matmul(psrot[:], rot_sb[:], raw[:], start=True, stop=True)
                sprod = rope_tmp.tile([HD, TC], F32, name="sprod", tag="ropetmp")
                nc.vector.tensor_tensor(sprod[:], psrot[:], sin_c[:], op=OP.mult)
                cprod = rope_tmp.tile([HD, TC], F32, name="cprod", tag="ropetmp")
                nc.vector.tensor_tensor(cprod[:], raw[:], cos_c[:], op=OP.mult)
                nc.gpsimd.tensor_tensor(
                    out[:, c0 : c0 + TC], cprod[:], sprod[:], op=OP.add
                )

        # ---- stage 2 block: attention for one (batch, query-chunk) ----
        def qc_block(b, qc):
            t0 = b * S
            q0 = t0 + QC * qc
            psc = [
                ps_b.tile([D + 1, QC], F32, name=f"psctx{h}", tag="ps_b")
                for h in range(HPC)
            ]
            for kt in range(NKT):
                k0 = t0 + 128 * kt
                # both heads' scores into one 2-bank psum tile -> single exp
                pss = ps_a.tile([128, HPC * QC], F32, name="pss", tag="ps_a")
                for h in range(HPC):
                    nc.tensor.matmul(
                        pss[:, QC * h : QC * (h + 1)],
                        k_rope[D * h : D * (h + 1), k0 : k0 + 128],
                        q_rope[D * h : D * (h + 1), q0 : q0 + QC],
                        start=True, stop=True,
                    )
                ex = exps_pool.tile([128, HPC * QC], dt_in, name="ex", tag="exps")
                nc.scalar.activation(ex[:], pss[:], AF.Exp, scale=scale)
                for h in range(HPC):
                    nc.tensor.matmul(
                        psc[h][:],
                        v_sb[k0 // 128][:, (D + 1) * h : (D + 1) * (h + 1)],
                        ex[:, QC * h : QC * (h + 1)],
                        start=(kt == 0), stop=(kt == NKT - 1),
                    )
            # normalize: 1/Z on DVE, broadcast via DRAM bounce, multiply
            for h in range(HPC):
                zr = zr_pool.tile([1, QC], F32, name="zrec", tag="zr")
                nc.vector.reciprocal(zr[:], psc[h][D : D + 1, :])
                zd = dram.tile([QC], F32, name="zd", tag="zd")
                nc.gpsimd.dma_start(zd[:], zr[:])
                zb = zb_pool.tile([D, QC], F32, name="zb", tag="zb")
                zd_ap = zd[:]
                nc.gpsimd.dma_start(
                    zb[:],
                    bass.AP(
                        tensor=zd_ap.tensor, offset=zd_ap.offset,
                        ap=[[0, D], [1, QC]],
                    ),
                )
                if h == 0:
                    nc.vector.tensor_tensor(
                        ctx_pack[b][0:D, QC * qc : QC * (qc + 1)],
                        psc[h][0:D, :], zb[:], op=OP.mult,
                    )
                else:
                    csh = csh_pool.tile([D, QC], dt_in, name="csh", tag="csh")
                    nc.vector.tensor_tensor(csh[:], psc[h][0:D, :], zb[:], op=OP.mult)
                    nc.scalar.dma_start(
                        ctx_pack[b][D : 2 * D, QC * qc : QC * (qc + 1)], csh[:]
                    )

        # ---- stage 3: output projection for some query tiles of batch b ----
        def emit_out_proj(b, tiles):
            t0 = b * S
            for j in tiles:
                for e in range(E // 512):
                    pso = ps_b.tile([128, 512], F32, name="pso", tag="ps_b")
                    nc.tensor.matmul(
                        pso[:],
                        ctx_pack[b][:, 128 * j : 128 * (j + 1)],
                        wo_sb[:, 512 * e : 512 * (e + 1)],
                        start=True, stop=True,
                    )
                    osb = osb_pool.tile([128, 512], F32, name="osb", tag="osb")
                    nc.vector.tensor_copy(osb[:], pso[:])
                    eng = nc.sync if e == 0 else nc.gpsimd
                    eng.dma_start(
                        yp.ap()[t0 + 128 * j : t0 + 128 * (j + 1), 512 * e : 512 * (e + 1)],
                        osb[:],
                    )

        # ---- interleaved emission ----
        # batch-0 projections; then batch-1 projections interleaved with
        # batch-0 attention; then batch-1 attention interleaved with batch-0
        # output projection; then batch-1 output projection.
        JT = S // 128  # out-proj tiles per batch
        NCB = NCH // B  # projection chunks per batch
        if NCB >= 1 and NQC >= 1 and NCH == B * NCB:
            for c in range(NCB):
                proj_chunk(c)
            for i in range(NCB):
                proj_chunk(NCB + i)
                if i < NQC:
                    qc_block(0, i)
            for qc in range(NCB, NQC):
                qc_block(0, qc)
            for qc in range(NQC):
                emit_out_proj(0, range(JT * qc // NQC, JT * (qc + 1) // NQC))
                qc_block(1, qc)
            emit_out_proj(1, range(JT))
        else:
            for c in range(NCH):
                proj_chunk(c)
            for b in range(B):
                for qc in range(NQC):
                    qc_block(b, qc)
            for b in range(B):
                emit_out_proj(b, range(JT))

    nc.compile()
    return nc


def _rope_tables():
    inv_freq = 1.0 / (10000.0 ** (np.arange(0, D, 2, dtype=np.float32) / D))
    t = np.arange(S, dtype=np.float32)
    freqs = np.outer(t, inv_freq).astype(np.float32)
    emb = np.concatenate([freqs, freqs], axis=-1)
    return np.cos(emb).astype(np.float32), np.sin(emb).astype(np.float32)


def _rot_matrix():
    R = np.zeros((HD, HD), np.float32)
    for hh in range(HPC):
        for do in range(D):
            po = D * hh + do
            if do < D // 2:
                R[D * hh + do + D // 2, po] = -1.0
            else:
                R[D * hh + do - D // 2, po] = 1.0
    return R


def kernel(x, Wq, bq, Wk, bk, Wv, bv, Wo, bo):
    global LAST_RESULTS
    import ml_dtypes

    x = np.asarray(x, dtype=np.float32)
    Wq, bq = np.asarray(Wq, np.float32), np.asarray(bq, np.float32)
    Wk, bk = np.asarray(Wk, np.float32), np.asarray(bk, np.float32)
    Wv, bv = np.asarray(Wv, np.float32), np.asarray(bv, np.float32)
    Wo, bo = np.asarray(Wo, np.float32), np.asarray(bo, np.float32)

    mode = MM_MODE
    dt_np = ml_dtypes.bfloat16 if mode == "bf16" else np.float32
    T = B * S

    if mode not in _NC_CACHE:
        _NC_CACHE[mode] = build_mha_nc(mode)
    nc = _NC_CACHE[mode]

    xT = np.ascontiguousarray(x.reshape(T, E).T).astype(dt_np)
    cos, sin = _rope_tables()
    cosT = np.tile(np.ascontiguousarray(cos.T), (HPC, B)).astype(np.float32)
    sinT = np.tile(np.ascontiguousarray(sin.T), (HPC, B)).astype(np.float32)
    R = _rot_matrix().astype(dt_np)

    in_maps = []
    for c in range(N_CORES):
        sl = slice(HD * c, HD * (c + 1))
        in_maps.append(
            {
                "xT": xT,
                "wq": np.ascontiguousarray(Wq[:, sl]).astype(dt_np),
                "wk": np.ascontiguousarray(Wk[:, sl]).astype(dt_np),
                "wv": np.ascontiguousarray(Wv[:, sl]).astype(dt_np),
                "bq": np.ascontiguousarray(bq[sl][:, None]).astype(np.float32),
                "bk": np.ascontiguousarray(bk[sl][:, None]).astype(np.float32),
                "bv": np.ascontiguousarray(bv[sl][:, None]).astype(np.float32),
                "wo": np.ascontiguousarray(Wo[sl, :]).astype(dt_np),
                "cosT": cosT,
                "sinT": sinT,
                "rot": R,
                "ones": np.ones((1, 1), dt_np),
            }
        )

    res = bass_utils.run_bass_kernel_spmd(nc, in_maps, core_ids=list(range(N_CORES)))
    LAST_RESULTS = res

    out = np.zeros((T, E), np.float64)
    for c in range(N_CORES):
        out += res.results[c]["yp"].astype(np.float64)
    out += bo.astype(np.float64)
    return out.astype(np.float32).reshape(B, S, E)
